# revision 1
# baseline (speedup 1.0000x reference)
"""BiATT kernel for 8 Trainium2 NeuronCores.

The reference module's bilinear-attention branch is dead code: the
"attention" weights are softmax(axis=1) over [N, 1] tensors, which is
exactly 1.0 for every row.  Hence

    cf_final = atoms_vector @ (Wcc[0:D] + Wcc[D:2D] + Wcc[2D:3D] + Wcc[3D:4D]) + bcc
    pf_final = amino_vector @ (Wcp[0:D] + Wcp[D:2D] + Wcp[2D:3D] + Wcp[3D:4D]) + bcp

bit-for-bit up to fp32 rounding.  The device kernel therefore computes two
[768, 512] @ [512, 512] matmuls per core (rows sharded 8 ways, folded
weights replicated).

Numerics: the default path splits each fp32 operand into bf16 hi + lo
halves and accumulates the three significant cross products in fp32 PSUM
(x@W = xh@Wh + xl@Wh + xh@Wl, the dropped xl@Wl term is ~2^-18).  Measured
end-to-end error vs the fp32 reference is ~5e-6, at 1/3 the PE cost and
the same DMA bytes as native fp32 matmuls.  BIATT_MM={raw,bf16x2,f32,f32r}
selects the scheme; the default "raw" is the same bf16x2 math on a
hand-scheduled (non-Tile) pipeline with a term-major matmul order.

Layout: rows of the shard live on PSUM partitions; the stationary matmul
operand is the pre-transposed activation row-block (host supplies
partition-major K-chunked arrays so every DMA is a large contiguous
transfer), the moving operand is the folded weight.  Input DMAs ride the
Sync HWDGE ring, output DMAs the Activation ring.  The bias is added on
the host during the gather (it is a rank-1 epilogue on the full output).
"""

import os

import ml_dtypes
import numpy as np

import concourse.bacc as bacc
import concourse.bass as bass
import concourse.mybir as mybir
import concourse.tile as tile
from concourse.bass_utils import run_bass_kernel_spmd

N_CORES = 8
D = 512          # feature dim
N_ROWS = 6144    # rows of atoms_vector / amino_vector
SHARD = N_ROWS // N_CORES   # 768 rows per core
P = 128          # SBUF partitions
KC = D // P      # 4 contraction chunks
NRB = SHARD // P  # 6 row blocks per shard

_F32 = mybir.dt.float32
_BF16 = mybir.dt.bfloat16
_PROGRAM_CACHE = {}

_LAST_EXEC_NS = None


def _new_bass():
    return bacc.Bacc(
        "TRN2",
        target_bir_lowering=False,
        debug=False,
        num_devices=N_CORES,
    )


def _build_bf16x2():
    """Split-bf16 path: per stream (cc / cp) the activation comes as hi/lo
    bf16 halves and the folded weight as hi/lo bf16 halves.  Input tensors
    are partition-major K-chunked ([128, nk, len]) so each is one large
    contiguous DMA.  psum[rb] accumulates 12 matmuls: k0..3 of xh@wh,
    xl@wh, xh@wl.

    Perf structure: inputs are two-chunk halves loaded in consumption order
    on the Sync HWDGE ring (output DMAs ride the Activation ring so the two
    dispatch streams never serialize against each other); a burst of
    throwaway matmuls on scratch tiles keeps the PE busy during the DMA
    lead so the HAM clock gate is released (2.4 GHz) when the real matmul
    stream starts."""
    nc = _new_bass()

    # names: {tensor}{piece}; each tensor comes as 2 two-chunk halves.
    d = {}
    layout = {}
    for t, ln, npiece, nk in (
        ("xh", SHARD, 2, 2), ("wcch", D, 2, 2),
        ("xl", SHARD, 2, 2), ("wccl", D, 2, 2),
        ("yh", SHARD, 2, 2), ("wcph", D, 2, 2),
        ("yl", SHARD, 2, 2), ("wcpl", D, 2, 2),
    ):
        layout[t] = (ln, npiece, nk)
        for h in range(npiece):
            d[f"{t}{h}"] = nc.dram_tensor(
                f"{t}{h}", [P, nk, ln], _BF16, kind="ExternalInput"
            ).ap()

    cf = nc.dram_tensor("cf", [NRB, P, D], _F32, kind="ExternalOutput").ap()
    pf = nc.dram_tensor("pf", [NRB, P, D], _F32, kind="ExternalOutput").ap()

    with tile.TileContext(nc) as tc:
        with (
            tc.tile_pool(name="ins", bufs=1) as ins,
            tc.tile_pool(name="warm", bufs=1) as warm,
            tc.tile_pool(name="psum", bufs=7, space=bass.MemorySpace.PSUM) as psum,
            tc.tile_pool(name="wpsum", bufs=1, space=bass.MemorySpace.PSUM) as wpsum,
            tc.tile_pool(name="outs", bufs=8) as outs,
        ):
            # PE warm-up: ~4us of dependency-free matmuls on scratch data,
            # issued while the input DMAs stream in.  Keeps the HAM activity
            # window busy so the real matmuls run at 2.4 GHz from the start.
            wsrc = warm.tile([P, 2 * P], _BF16, tag="wsrc")
            nc.gpsimd.memset(wsrc[:], 0.0)
            wps = wpsum.tile([P, P], _F32, tag="wps")
            for i in range(40):
                nc.tensor.matmul(
                    wps[:], wsrc[:, 0:P], wsrc[:, P:2 * P],
                    start=(i == 0), stop=(i == 39),
                )

            # Load order == consumption order (cf stream first).
            s = {}
            def load(engine, name):
                ln, npiece, nk = layout[name[:-1]]
                t = ins.tile([P, nk, ln], _BF16, tag=name)
                engine.dma_start(t[:], d[name][:])
                s[name] = t

            for name in ("wcch0", "xh0", "wcch1", "xh1",
                         "xl0", "xl1", "wccl0", "wccl1",
                         "wcph0", "yh0", "wcph1", "yh1",
                         "yl0", "yl1", "wcpl0", "wcpl1"):
                load(nc.sync, name)

            def piece(t, k):
                ln, npiece, nk = layout[t]
                return s[f"{t}{k // nk}"][:, k % nk, :]

            for a, w, out_d in (("x", "wcc", cf), ("y", "wcp", pf)):
                for rb in range(NRB):
                    ps = psum.tile([P, D], _F32, tag="ps")
                    idx = 0
                    for ah, wh2 in ((f"{a}h", f"{w}h"), (f"{a}l", f"{w}h"),
                                    (f"{a}h", f"{w}l")):
                        for k in range(KC):
                            nc.tensor.matmul(
                                ps[:],
                                piece(ah, k)[:, rb * P:(rb + 1) * P],
                                piece(wh2, k),
                                start=(idx == 0),
                                stop=(idx == 3 * KC - 1),
                            )
                            idx += 1
                    ot = outs.tile([P, D], _F32, tag="ot")
                    nc.vector.tensor_copy(ot[:], ps[:])
                    nc.scalar.dma_start(out_d[rb], ot[:])

    nc.compile()
    return nc


_IN_ORDER = ("wcch0", "xh0", "wcch1", "xh1", "xl0", "xl1", "wccl0", "wccl1",
             "wcph0", "yh0", "wcph1", "yh1", "yl0", "yl1", "wcpl0", "wcpl1")


def _build_raw():
    """Same bf16x2 math as _build_bf16x2 but hand-scheduled raw bacc: four
    semaphores pipeline input-DMAs (Sync ring) -> matmuls (PE) -> PSUM
    copies (DVE) -> output-DMAs (Activation ring).  Avoids the Tile
    framework's entry barrier and exit semaphore-reset butterfly (~14us).

    Static schedule: group g (0-5 = cf row-blocks, 6-11 = pf row-blocks)
    accumulates its 12 matmuls into PSUM bank g%8; groups g>=8 wait for the
    DVE copy of group g-8 before touching the recycled bank (also keeps the
    fatal same-bank PE-write/DVE-read overlap impossible).  DMA completions
    on one ring are NOT FIFO (each DMA fans out over the 16 SDMA engines),
    so each matmul term's input set gets its own semaphore with an
    all-members threshold instead of prefix counts on a shared one."""
    from contextlib import ExitStack

    nc = _new_bass()

    # Every tensor comes as two two-chunk halves — large per-partition
    # lines DMA at full rate, and finer splits measured as a net loss
    # (longer dispatch tail delays the later input gates).
    d = {}
    layout = {}
    for t, ln, npiece, nk in (
        ("xh", SHARD, 2, 2), ("wcch", D, 2, 2),
        ("xl", SHARD, 2, 2), ("wccl", D, 2, 2),
        ("yh", SHARD, 2, 2), ("wcph", D, 2, 2),
        ("yl", SHARD, 2, 2), ("wcpl", D, 2, 2),
    ):
        layout[t] = (ln, npiece, nk)
        for h in range(npiece):
            d[f"{t}{h}"] = nc.dram_tensor(
                f"{t}{h}", [P, nk, ln], _BF16, kind="ExternalInput"
            ).ap()
    cf = nc.dram_tensor("cf", [NRB, P, D], _F32, kind="ExternalOutput").ap()
    pf = nc.dram_tensor("pf", [NRB, P, D], _F32, kind="ExternalOutput").ap()

    NWARM = 40
    NOUT = 6  # SBUF output staging slots

    with ExitStack() as ctx:
        sb = {
            name: ctx.enter_context(
                nc.sbuf_tensor(
                    f"sb_{name}",
                    [P, layout[name[:-1]][2], layout[name[:-1]][0]],
                    _BF16,
                )
            )
            for name in _IN_ORDER
        }
        outsb = [
            ctx.enter_context(nc.sbuf_tensor(f"outsb{i}", [P, D], _F32))
            for i in range(NOUT)
        ]
        warm = ctx.enter_context(nc.sbuf_tensor("warmsb", [P, 2 * P], _BF16))
        ps = [
            ctx.enter_context(nc.psum_tensor(f"psum{i}", [P, D], _F32))
            for i in range(8)
        ]
        s_mm = ctx.enter_context(nc.semaphore("s_mm"))
        s_cp = ctx.enter_context(nc.semaphore("s_cp"))
        s_wm = ctx.enter_context(nc.semaphore("s_wm"))
        # Per-staging-slot output-DMA completion sems (a shared counter
        # would race: DMA completions are not FIFO across in-flight DMAs).
        s_ot = [
            ctx.enter_context(nc.semaphore(f"s_ot{i}")) for i in range(NOUT)
        ]
        # One semaphore per matmul-term input set; threshold = 16 * |set|.
        # The cf hi-term gates are per K-chunk so the first matmuls start
        # as soon as the first two DMAs land.
        gate_members = {
            "cfh0": ("wcch0", "xh0"), "cfh1": ("wcch1", "xh1"),
            "cfl": ("xl0", "xl1"),
            "cfw": ("wccl0", "wccl1"),
            "pfh0": ("wcph0", "yh0"), "pfh1": ("wcph1", "yh1"),
            "pfl": ("yl0", "yl1"),
            "pfw": ("wcpl0", "wcpl1"),
        }
        gates = {
            gn: ctx.enter_context(nc.semaphore(f"s_{gn}"))
            for gn in gate_members
        }
        sem_of = {}
        for gn, members in gate_members.items():
            for name in members:
                sem_of[name] = gates[gn]

        def piece(t, k):
            nk = layout[t][2]
            return sb[f"{t}{k // nk}"][:, k % nk, :]

        def groups():
            for gi, (a, w) in enumerate((("x", "wcc"), ("y", "wcp"))):
                for rb in range(NRB):
                    yield gi * NRB + rb, a, w, rb

        with nc.Block() as block:

            @block.sync
            def _(sync):
                for name in _IN_ORDER:
                    sync.dma_start(sb[name][:], d[name][:]).then_inc(
                        sem_of[name], 16
                    )

            @block.gpsimd
            def _(gpsimd):
                nc.gpsimd.memset(warm[:], 0.0).then_inc(s_wm, 1)

            @block.tensor
            def _(tensor):
                # HAM warm-up on scratch data (bank 7 is reset by group 7's
                # start=True before anything reads it).
                tensor.wait_ge(s_wm, 1)
                for i in range(NWARM):
                    nc.tensor.matmul(
                        ps[7][:, 0:P], warm[:, 0:P], warm[:, P:2 * P],
                        start=(i == 0), stop=(i == NWARM - 1),
                    )
                waited = set()

                def gate(gn):
                    if gn not in waited:
                        waited.add(gn)
                        tensor.wait_ge(gates[gn], 16 * len(gate_members[gn]))

                # Term-major order per stream: all hi@Wh matmuls for the six
                # row-blocks first (they only need the first input pair),
                # then lo@Wh, then hi@Wl — so input DMAs stream in behind a
                # stall-free PE.  Phases A/B iterate k-outer (finer gate
                # granularity); phase C iterates rb-outer so the six groups
                # finish staggered and copies/output DMAs overlap the rest.
                for a, w, gbase, pfx in (("x", "wcc", 0, "cf"),
                                         ("y", "wcp", NRB, "pf")):
                    terms = ((f"{a}h", f"{w}h"), (f"{a}l", f"{w}h"),
                             (f"{a}h", f"{w}l"))
                    for ti in (0, 1):
                        ah, wh2 = terms[ti]
                        for k in range(KC):
                            gate(f"{pfx}h{k // 2}" if ti == 0 else f"{pfx}l")
                            for rb in range(NRB):
                                g = gbase + rb
                                if ti == 0 and k == 0 and g >= 8:
                                    tensor.wait_ge(s_cp, g - 7)
                                nc.tensor.matmul(
                                    ps[g % 8][:],
                                    piece(ah, k)[:, rb * P:(rb + 1) * P],
                                    piece(wh2, k),
                                    start=(ti == 0 and k == 0),
                                    stop=False,
                                )
                    ah, wh2 = terms[2]
                    gate(f"{pfx}w")
                    for rb in range(NRB):
                        g = gbase + rb
                        last = None
                        for k in range(KC):
                            last = nc.tensor.matmul(
                                ps[g % 8][:],
                                piece(ah, k)[:, rb * P:(rb + 1) * P],
                                piece(wh2, k),
                                start=False,
                                stop=(k == KC - 1),
                            )
                        last.then_inc(s_mm, 1)

            # The final group is copied and stored in two half-width pieces
            # so the second half's DMA overlaps the first's — it is the only
            # copy+store pair on the critical path.
            LAST = 2 * NRB - 1
            H = D // 2

            @block.vector
            def _(vector):
                for g in range(2 * NRB):
                    vector.wait_ge(s_mm, g + 1)
                    if g >= NOUT:
                        vector.wait_ge(s_ot[g % NOUT], 16 * (g // NOUT))
                    if g == LAST:
                        for h in range(2):
                            nc.vector.tensor_copy(
                                outsb[g % NOUT][:, h * H:(h + 1) * H],
                                ps[g % 8][:, h * H:(h + 1) * H],
                            ).then_inc(s_cp, 1)
                    else:
                        nc.vector.tensor_copy(
                            outsb[g % NOUT][:], ps[g % 8][:]
                        ).then_inc(s_cp, 1)

            @block.scalar
            def _(scalar):
                for g in range(2 * NRB):
                    out_d = cf if g < NRB else pf
                    if g == LAST:
                        for h in range(2):
                            scalar.wait_ge(s_cp, g + 1 + h)
                            scalar.dma_start(
                                out_d[g % NRB][:, h * H:(h + 1) * H],
                                outsb[g % NOUT][:, h * H:(h + 1) * H],
                            ).then_inc(s_ot[g % NOUT], 16)
                    else:
                        scalar.wait_ge(s_cp, g + 1)
                        scalar.dma_start(
                            out_d[g % NRB], outsb[g % NOUT][:]
                        ).then_inc(s_ot[g % NOUT], 16)

        nc.compile()
    return nc


def _build_f32(mm_dtype):
    """Single-dtype path (f32 or f32r), same layout as bf16x2 but one term."""
    nc = _new_bass()

    d = {}
    for t, ln in (("x", SHARD), ("y", SHARD), ("wcc", D), ("wcp", D)):
        for h in range(2):
            d[f"{t}{h}"] = nc.dram_tensor(
                f"{t}{h}", [P, 2, ln], mm_dtype, kind="ExternalInput"
            ).ap()

    cf = nc.dram_tensor("cf", [NRB, P, D], _F32, kind="ExternalOutput").ap()
    pf = nc.dram_tensor("pf", [NRB, P, D], _F32, kind="ExternalOutput").ap()

    with tile.TileContext(nc) as tc:
        with (
            tc.tile_pool(name="ins", bufs=1) as ins,
            tc.tile_pool(name="psum", bufs=8, space=bass.MemorySpace.PSUM) as psum,
            tc.tile_pool(name="outs", bufs=8) as outs,
        ):
            s = {}
            for name, ln in (
                ("wcc0", D), ("x0", SHARD), ("wcc1", D), ("x1", SHARD),
                ("wcp0", D), ("y0", SHARD), ("wcp1", D), ("y1", SHARD),
            ):
                t = ins.tile([P, 2, ln], mm_dtype, tag=name)
                nc.sync.dma_start(t[:], d[name][:])
                s[name] = t

            for a, w, out_d in (("x", "wcc", cf), ("y", "wcp", pf)):
                for rb in range(NRB):
                    ps = psum.tile([P, D], _F32, tag="ps")
                    for k in range(KC):
                        nc.tensor.matmul(
                            ps[:],
                            s[f"{a}{k // 2}"][:, k % 2, rb * P:(rb + 1) * P],
                            s[f"{w}{k // 2}"][:, k % 2, :],
                            start=(k == 0),
                            stop=(k == KC - 1),
                        )
                    ot = outs.tile([P, D], _F32, tag="ot")
                    nc.vector.tensor_copy(ot[:], ps[:])
                    nc.scalar.dma_start(out_d[rb], ot[:])

    nc.compile()
    return nc


def _get_program(scheme):
    if scheme not in _PROGRAM_CACHE:
        if scheme == "raw":
            _PROGRAM_CACHE[scheme] = _build_raw()
        elif scheme == "bf16x2":
            _PROGRAM_CACHE[scheme] = _build_bf16x2()
        else:
            _PROGRAM_CACHE[scheme] = _build_f32(
                mybir.dt.float32r if scheme == "f32r" else _F32
            )
    return _PROGRAM_CACHE[scheme]


def _chunk_pieces(mat_t, dtype, npiece):
    """[K=512, len] -> npiece contiguous [128, 4/npiece, len] partition-major
    K-chunk groups."""
    ln = mat_t.shape[1]
    c = np.ascontiguousarray(
        mat_t.reshape(KC, P, ln).transpose(1, 0, 2).astype(dtype)
    )  # [128, 4, len]
    per = KC // npiece
    return [np.ascontiguousarray(c[:, i * per:(i + 1) * per]) for i in range(npiece)]


def _chunk_halves(mat_t, dtype):
    return _chunk_pieces(mat_t, dtype, 2)


def _split_hi_lo(a):
    hi = a.astype(ml_dtypes.bfloat16)
    lo = (a - hi.astype(np.float32)).astype(ml_dtypes.bfloat16)
    return hi, lo


def kernel(**inputs):
    global _LAST_EXEC_NS

    atoms = np.ascontiguousarray(np.asarray(inputs["atoms_vector"], dtype=np.float32))
    amino = np.ascontiguousarray(np.asarray(inputs["amino_vector"], dtype=np.float32))
    Wcc = np.asarray(inputs["Wcc"], dtype=np.float32)
    Wcp = np.asarray(inputs["Wcp"], dtype=np.float32)
    bcc = np.asarray(inputs["bcc"], dtype=np.float32)
    bcp = np.asarray(inputs["bcp"], dtype=np.float32)

    # Fold the four weight blocks (concat([v]*4, 1) @ W == v @ sum-of-blocks).
    wcc_f = Wcc.reshape(4, D, D).sum(axis=0)
    wcp_f = Wcp.reshape(4, D, D).sum(axis=0)

    scheme = os.environ.get("BIATT_MM", "raw")
    nc = _get_program(scheme)

    in_maps = []
    if scheme in ("bf16x2", "raw"):
        # raw: wcch/xh in four per-chunk pieces, the rest in two halves;
        # tile bf16x2: everything in two halves.
        n_first = 2
        wcch, wccl = _split_hi_lo(wcc_f)
        wcph, wcpl = _split_hi_lo(wcp_f)
        w_parts = {}
        for nm, arr, npiece in (("wcch", wcch, n_first), ("wccl", wccl, 2),
                                ("wcph", wcph, 2), ("wcpl", wcpl, 2)):
            for i, p in enumerate(_chunk_pieces(arr, ml_dtypes.bfloat16, npiece)):
                w_parts[f"{nm}{i}"] = p
        for c in range(N_CORES):
            sl = slice(c * SHARD, (c + 1) * SHARD)
            m = dict(w_parts)
            for nm, base in (("x", atoms), ("y", amino)):
                t = base[sl].T  # [512, 768]
                hi, lo = _split_hi_lo(t)
                nh = n_first if nm == "x" else 2
                for i, p in enumerate(_chunk_pieces(hi, ml_dtypes.bfloat16, nh)):
                    m[f"{nm}h{i}"] = p
                for i, p in enumerate(_chunk_pieces(lo, ml_dtypes.bfloat16, 2)):
                    m[f"{nm}l{i}"] = p
            in_maps.append(m)
    else:
        w_parts = {}
        for nm, arr in (("wcc", wcc_f), ("wcp", wcp_f)):
            w_parts[f"{nm}0"], w_parts[f"{nm}1"] = _chunk_halves(arr, np.float32)
        for c in range(N_CORES):
            sl = slice(c * SHARD, (c + 1) * SHARD)
            m = dict(w_parts)
            m["x0"], m["x1"] = _chunk_halves(atoms[sl].T, np.float32)
            m["y0"], m["y1"] = _chunk_halves(amino[sl].T, np.float32)
            in_maps.append(m)

    trace = bool(os.environ.get("BIATT_TRACE"))
    try:
        res = run_bass_kernel_spmd(nc, in_maps, list(range(N_CORES)), trace=trace)
    except Exception:
        # One retry: a transiently wedged NeuronCore surfaces as a runtime
        # error on an otherwise-valid program.
        res = run_bass_kernel_spmd(nc, in_maps, list(range(N_CORES)), trace=trace)
    _LAST_EXEC_NS = res.exec_time_ns

    cf = np.concatenate(
        [res.results[c]["cf"].reshape(SHARD, D) for c in range(N_CORES)], axis=0
    )
    pf = np.concatenate(
        [res.results[c]["pf"].reshape(SHARD, D) for c in range(N_CORES)], axis=0
    )
    cf += bcc  # rank-1 epilogue on the gathered output
    pf += bcp
    return cf, pf



# revision 34
# speedup vs baseline: 2.9314x; 2.9314x over previous
"""BiATT kernel for 8 Trainium2 NeuronCores.

The reference module's bilinear-attention branch is dead code: the
"attention" weights are softmax(axis=1) over [N, 1] tensors, which is
exactly 1.0 for every row.  Hence

    cf_final = atoms_vector @ (Wcc[0:D] + Wcc[D:2D] + Wcc[2D:3D] + Wcc[3D:4D]) + bcc
    pf_final = amino_vector @ (Wcp[0:D] + Wcp[D:2D] + Wcp[2D:3D] + Wcp[3D:4D]) + bcp

bit-for-bit up to fp32 rounding, i.e. two [6144,512] @ [512,512] matmuls
with host-folded weights.

Default scheme "bf16s": stream-split sharding (cores 0-3 compute cf rows,
cores 4-7 pf rows, 1536 rows per core) with a SINGLE bf16 matmul term and
bf16 outputs, converted to f32 on the host.  Measured end-to-end error is
~3.5e-3 against the fp32 reference (the harness gate is 2e-2); per-core
traffic is 2.05 MB in + 1.57 MB out and 48 matmuls (805 MFLOP) -- one
third of the PE work and a quarter of the DMA bytes of the bf16x2 path.
See _build_bf16s for the pipeline schedule.  BIATT_MM selects the legacy
schemes (raw = hand-scheduled bf16x2, bf16x2 = Tile bf16x2, f32, f32r);
BIATT_NWARM tunes the warm-up burst.

The bias is added on the host during the gather (a rank-1 epilogue on the
full output).
"""

import os

import ml_dtypes
import numpy as np

import concourse.bacc as bacc
import concourse.bass as bass
import concourse.mybir as mybir
import concourse.tile as tile
from concourse.bass_utils import run_bass_kernel_spmd

N_CORES = 8
D = 512          # feature dim
N_ROWS = 6144    # rows of atoms_vector / amino_vector
SHARD = N_ROWS // N_CORES   # 768 rows per core
P = 128          # SBUF partitions
KC = D // P      # 4 contraction chunks
NRB = SHARD // P  # 6 row blocks per shard

_F32 = mybir.dt.float32
_BF16 = mybir.dt.bfloat16
_PROGRAM_CACHE = {}

_LAST_EXEC_NS = None


def _new_bass():
    return bacc.Bacc(
        "TRN2",
        target_bir_lowering=False,
        debug=False,
        num_devices=N_CORES,
    )


def _build_bf16x2():
    """Split-bf16 path: per stream (cc / cp) the activation comes as hi/lo
    bf16 halves and the folded weight as hi/lo bf16 halves.  Input tensors
    are partition-major K-chunked ([128, nk, len]) so each is one large
    contiguous DMA.  psum[rb] accumulates 12 matmuls: k0..3 of xh@wh,
    xl@wh, xh@wl.

    Perf structure: inputs are two-chunk halves loaded in consumption order
    on the Sync HWDGE ring (output DMAs ride the Activation ring so the two
    dispatch streams never serialize against each other); a burst of
    throwaway matmuls on scratch tiles keeps the PE busy during the DMA
    lead so the HAM clock gate is released (2.4 GHz) when the real matmul
    stream starts."""
    nc = _new_bass()

    # names: {tensor}{piece}; each tensor comes as 2 two-chunk halves.
    d = {}
    layout = {}
    for t, ln, npiece, nk in (
        ("xh", SHARD, 2, 2), ("wcch", D, 2, 2),
        ("xl", SHARD, 2, 2), ("wccl", D, 2, 2),
        ("yh", SHARD, 2, 2), ("wcph", D, 2, 2),
        ("yl", SHARD, 2, 2), ("wcpl", D, 2, 2),
    ):
        layout[t] = (ln, npiece, nk)
        for h in range(npiece):
            d[f"{t}{h}"] = nc.dram_tensor(
                f"{t}{h}", [P, nk, ln], _BF16, kind="ExternalInput"
            ).ap()

    cf = nc.dram_tensor("cf", [NRB, P, D], _F32, kind="ExternalOutput").ap()
    pf = nc.dram_tensor("pf", [NRB, P, D], _F32, kind="ExternalOutput").ap()

    with tile.TileContext(nc) as tc:
        with (
            tc.tile_pool(name="ins", bufs=1) as ins,
            tc.tile_pool(name="warm", bufs=1) as warm,
            tc.tile_pool(name="psum", bufs=7, space=bass.MemorySpace.PSUM) as psum,
            tc.tile_pool(name="wpsum", bufs=1, space=bass.MemorySpace.PSUM) as wpsum,
            tc.tile_pool(name="outs", bufs=8) as outs,
        ):
            # PE warm-up: ~4us of dependency-free matmuls on scratch data,
            # issued while the input DMAs stream in.  Keeps the HAM activity
            # window busy so the real matmuls run at 2.4 GHz from the start.
            wsrc = warm.tile([P, 2 * P], _BF16, tag="wsrc")
            nc.gpsimd.memset(wsrc[:], 0.0)
            wps = wpsum.tile([P, P], _F32, tag="wps")
            for i in range(40):
                nc.tensor.matmul(
                    wps[:], wsrc[:, 0:P], wsrc[:, P:2 * P],
                    start=(i == 0), stop=(i == 39),
                )

            # Load order == consumption order (cf stream first).
            s = {}
            def load(engine, name):
                ln, npiece, nk = layout[name[:-1]]
                t = ins.tile([P, nk, ln], _BF16, tag=name)
                engine.dma_start(t[:], d[name][:])
                s[name] = t

            for name in ("wcch0", "xh0", "wcch1", "xh1",
                         "xl0", "xl1", "wccl0", "wccl1",
                         "wcph0", "yh0", "wcph1", "yh1",
                         "yl0", "yl1", "wcpl0", "wcpl1"):
                load(nc.sync, name)

            def piece(t, k):
                ln, npiece, nk = layout[t]
                return s[f"{t}{k // nk}"][:, k % nk, :]

            for a, w, out_d in (("x", "wcc", cf), ("y", "wcp", pf)):
                for rb in range(NRB):
                    ps = psum.tile([P, D], _F32, tag="ps")
                    idx = 0
                    for ah, wh2 in ((f"{a}h", f"{w}h"), (f"{a}l", f"{w}h"),
                                    (f"{a}h", f"{w}l")):
                        for k in range(KC):
                            nc.tensor.matmul(
                                ps[:],
                                piece(ah, k)[:, rb * P:(rb + 1) * P],
                                piece(wh2, k),
                                start=(idx == 0),
                                stop=(idx == 3 * KC - 1),
                            )
                            idx += 1
                    ot = outs.tile([P, D], _F32, tag="ot")
                    nc.vector.tensor_copy(ot[:], ps[:])
                    nc.scalar.dma_start(out_d[rb], ot[:])

    nc.compile()
    return nc


_IN_ORDER = ("wcch0", "xh0", "wcch1", "xh1", "xl0", "xl1", "wccl0", "wccl1",
             "wcph0", "yh0", "wcph1", "yh1", "yl0", "yl1", "wcpl0", "wcpl1")

# ---------------------------------------------------------------------------
# Single-term bf16 scheme ("bf16s"): one stream per core (cores 0-3 compute
# cf rows, 4-7 pf rows; 1536 rows each), single bf16 matmul term (measured
# end-to-end error ~3.4e-3 vs the 2e-2 gate), bf16 outputs converted to f32
# on the host.  Per-core traffic: 2.05 MB in + 1.57 MB out vs the bf16x2
# path's 8.4 MB; PE work 48 matmuls (805 MFLOP) vs 144.
# ---------------------------------------------------------------------------

SHARD2 = N_ROWS // 4      # 1536 rows per core (4 cores per stream)
NRB2 = SHARD2 // P        # 12 row blocks
XH = SHARD2 // 2          # 768 columns per xT half (row-blocks 0-5 / 6-11)


# Output DMA plan: (blocks, queue) per DMA.  Early big pieces go on the
# Activation queue; the late pieces spread across queues so their
# HWDGE/SWDGE dispatch chains overlap.  "pool" DMAs use the SWDGE path,
# which does not occupy the shared HWDGE generator at all.
OUT_PLAN = ((3, "act"), (3, "act"), (2, "act"), (2, "act"), (1, "sync"),
            (1, "sync"))
OUT_SPLIT = tuple(n for n, _ in OUT_PLAN)


def _build_bf16s(nwarm=30, out_plan=OUT_PLAN):
    """Hand-scheduled raw pipeline, one [1536,512]@[512,512] bf16 matmul.

    DMA dispatch is the scarce resource (each HWDGE dispatch occupies the
    shared generator ~630ns, and the first DMA's SEQ+HWDGE+DGE lead is
    ~1.9us), so inputs are 7 DMAs: p0a/p0b split the k0 chunk (weight
    [128,512] packed with activation row-blocks 0-2 / 3-5) so the first
    matmul gates on 224KB; p1..p3 pack weight chunk k with activation
    chunk k of row-blocks 0-5; xb01/xb23 carry the activation chunks of
    row-blocks 6-11 two-at-a-time.  Outputs: row-blocks 0-10 are
    f32->bf16 copied (alternating DVE/GPSIMD, whose 658/806ns per block
    matches the 426ns stop stagger) into 4 private SBUF slots and stored
    by 4 Activation-queue DMAs (3/3/3/2 blocks, partition-major
    [128,12,512] DRAM, host re-transposes); block 11 is DMA'd from its
    PSUM bank as f32 on the idle sync queue, skipping the copy.

    The warm-up matmuls run on uninitialized SBUF from t~0 (their PSUM
    bank is reset by a later start=True) purely to hold the PE p-state at
    full clock before the real stream starts.

    Groups 0-5 (banks 0-5): k0,k1 k-outer, then per-rb (k2,k3) so stops
    stagger 426ns; groups 6-11 (banks 6,7,0-3) mirror, bank 0-3 reuse
    gated on the first-half copies."""
    from contextlib import ExitStack

    nc = _new_bass()

    d = {}
    d["p0a"] = nc.dram_tensor("p0a", [P, D + P], _BF16, kind="ExternalInput").ap()
    d["p0b"] = nc.dram_tensor("p0b", [P, 5 * P], _BF16, kind="ExternalInput").ap()
    for k in range(1, KC):
        d[f"p{k}"] = nc.dram_tensor(
            f"p{k}", [P, D + XH], _BF16, kind="ExternalInput").ap()
    d["xb01"] = nc.dram_tensor("xb01", [P, 2, XH], _BF16, kind="ExternalInput").ap()
    d["xb23"] = nc.dram_tensor("xb23", [P, 2, XH], _BF16, kind="ExternalInput").ap()
    out_d = nc.dram_tensor("out", [P, NRB2, D], _BF16, kind="ExternalOutput").ap()

    out_split = tuple(n for n, _ in out_plan)
    DMA_ORDER = ("p0a", "p0b", "p1", "p2", "p3", "xb01", "xb23")
    SHAPES = {"p0a": [P, D + P], "p0b": [P, 5 * P],
              "p1": [P, D + XH], "p2": [P, D + XH], "p3": [P, D + XH],
              "xb01": [P, 2, XH], "xb23": [P, 2, XH]}
    NJ = len(out_split)

    with ExitStack() as ctx:
        sb = {
            name: ctx.enter_context(
                nc.sbuf_tensor(f"sb_{name}", SHAPES[name], _BF16))
            for name in DMA_ORDER
        }
        # One private slot per output DMA -- no ping-pong waits anywhere.
        max_n = max(out_split)
        outsb = ctx.enter_context(
            nc.sbuf_tensor("outsb", [P, NJ, max_n * D], _BF16))
        warm = ctx.enter_context(nc.sbuf_tensor("warmsb", [P, 2 * P], _BF16))
        ps = [
            ctx.enter_context(nc.psum_tensor(f"psum{i}", [P, D], _F32))
            for i in range(8)
        ]
        s_mm = ctx.enter_context(nc.semaphore("s_mm"))
        s_cpv = ctx.enter_context(nc.semaphore("s_cpv"))  # DVE copies
        s_cpg = ctx.enter_context(nc.semaphore("s_cpg"))  # GPSIMD copies
        s_cpa = ctx.enter_context(nc.semaphore("s_cpa"))  # ACT copies
        s_od = ctx.enter_context(nc.semaphore("s_od"))    # out-DMA completions
        gates = {
            name: ctx.enter_context(nc.semaphore(f"s_{name}"))
            for name in DMA_ORDER
        }

        blk_dma = []
        for j, n in enumerate(out_split):
            for o in range(n):
                blk_dma.append((j, o))
        dma_first_blk = [sum(out_split[:j]) for j in range(NJ)]
        # 0 = DVE, 1 = GPSIMD, 2 = ACT (unused by default).
        cp_eng = [g % 2 for g in range(NRB2)]
        cp_sem_of = {0: s_cpv, 1: s_cpg, 2: s_cpa}

        def cp_counts(last_blk):
            return tuple(
                sum(1 for g in range(NRB2) if cp_eng[g] == e and g <= last_blk)
                for e in range(3)
            )

        def wchunk(k):
            return sb["p0a" if k == 0 else f"p{k}"][:, 0:D]

        def xblk(half, k, rb):
            if half == "a":
                if k == 0:
                    t, r = ("p0a", rb) if rb < 1 else ("p0b", rb - 1)
                    off = D if t == "p0a" else 0
                    return sb[t][:, off + r * P:off + (r + 1) * P]
                return sb[f"p{k}"][:, D + rb * P:D + (rb + 1) * P]
            return sb["xb01" if k < 2 else "xb23"][:, k % 2, rb * P:(rb + 1) * P]

        def copy_loop(eng_idx, engine, eng_ns, sem):
            for g in range(NRB2):
                if cp_eng[g] != eng_idx:
                    continue
                j, o = blk_dma[g]
                engine.wait_ge(s_mm, g + 1)
                eng_ns.tensor_copy(
                    outsb[:, j, o * D:(o + 1) * D], ps[g % 8][:]
                ).then_inc(sem, 1)

        with nc.Block() as block:

            def issue_out(handle, eng_ns, j):
                n = out_split[j]
                b0 = dma_first_blk[j]
                counts = cp_counts(b0 + n - 1)
                for e, cnt in enumerate(counts):
                    if cnt:
                        handle.wait_ge(cp_sem_of[e], cnt)
                eng_ns.dma_start(
                    out_d[:, b0:b0 + n, :], outsb[:, j, 0:n * D]
                ).then_inc(s_od, 16)

            def act_copy(scalar, g):
                j, o = blk_dma[g]
                scalar.wait_ge(s_mm, g + 1)
                nc.scalar.activation(
                    outsb[:, j, o * D:(o + 1) * D], ps[g % 8][:],
                    mybir.ActivationFunctionType.Copy,
                ).then_inc(s_cpa, 1)

            @block.sync
            def _(sync):
                for name in DMA_ORDER:
                    if name == "p0a":
                        continue  # issued on the Pool/SWDGE queue
                    sync.dma_start(sb[name][:], d[name][:]).then_inc(
                        gates[name], 16
                    )
                for j, (n, q) in enumerate(out_plan):
                    if q == "sync":
                        issue_out(sync, nc.sync, j)

            @block.vector
            def _(vector):
                copy_loop(0, vector, nc.vector, s_cpv)
                for j, (n, q) in enumerate(out_plan):
                    if q == "vec":
                        issue_out(vector, nc.vector, j)

            @block.gpsimd
            def _(gpsimd):
                nc.gpsimd.dma_start(sb["p0a"][:], d["p0a"][:]).then_inc(
                    gates["p0a"], 16
                )
                for j, (n, q) in enumerate(out_plan):
                    if q == "pool":
                        issue_out(gpsimd, nc.gpsimd, j)
                for j, (n, q) in enumerate(out_plan):
                    if q == "pool":
                        issue_out(gpsimd, nc.gpsimd, j)

            @block.tensor
            def _(tensor):
                for i in range(nwarm):
                    nc.tensor.matmul(
                        ps[7][:, 0:P], warm[:, 0:P], warm[:, P:2 * P],
                        start=(i == 0), stop=(i == nwarm - 1),
                    )
                waited = set()

                def gate(name):
                    if name not in waited:
                        waited.add(name)
                        tensor.wait_ge(gates[name], 16)

                def bank_wait(g):
                    if g >= 8:
                        e = cp_eng[g - 8]
                        tensor.wait_ge(cp_sem_of[e], cp_counts(g - 8)[e])

                def mm(half, k, rb, start, stop):
                    g = (0 if half == "a" else 6) + rb
                    if start:
                        bank_wait(g)
                    m = nc.tensor.matmul(
                        ps[g % 8][:], xblk(half, k, rb), wchunk(k),
                        start=start, stop=stop,
                    )
                    if stop:
                        m.then_inc(s_mm, 1)

                # Phase a: the first gate blocks the PE SEQ, and the ~5
                # instructions issued after any blocking wait run at the
                # mid p-state -- burn them on short throwaway matmuls so
                # every real matmul runs at full clock.
                gate("p0a")
                for i in range(5):
                    nc.tensor.matmul(
                        ps[7][:, 0:32], warm[:, 0:P], warm[:, P:P + 32],
                        start=(i == 0), stop=(i == 4),
                    )
                for rb in range(6):
                    if rb == 1:
                        gate("p0b")
                    mm("a", 0, rb, True, False)
                gate("p1")
                for rb in range(6):
                    mm("a", 1, rb, False, False)
                gate("p2")
                gate("p3")
                for rb in range(6):
                    mm("a", 2, rb, False, False)
                    mm("a", 3, rb, False, True)
                # Phase b: k0 k-outer; per-rb (k1,k2,k3) so stops stagger
                # 639ns apart, matching the DVE copy throughput.
                gate("xb01")
                for rb in range(6):
                    mm("b", 0, rb, True, False)
                for rb in range(6):
                    mm("b", 1, rb, False, False)
                    if rb == 0:
                        gate("xb23")
                    mm("b", 2, rb, False, False)
                    mm("b", 3, rb, False, True)

            @block.scalar
            def _(scalar):
                # Dummy activation so the Copy act-table is loaded long
                # before the first real copy.
                nc.scalar.activation(
                    outsb[:, 0, 0:32], warm[:, 0:32],
                    mybir.ActivationFunctionType.Copy,
                )
                # Interleave ACT copies and this queue's out-DMAs in
                # dependency order: copy g keyed g, DMA j keyed by its last
                # block + 0.5, so every DMA follows the copies it waits on.
                items = []
                for j, (n, q) in enumerate(out_plan):
                    if q == "act":
                        items.append((dma_first_blk[j] + n - 0.5, "dma", j))
                for g in range(NRB2):
                    if cp_eng[g] == 2:
                        items.append((float(g), "cp", g))
                for _, kind, idx in sorted(items):
                    if kind == "dma":
                        issue_out(scalar, nc.scalar, idx)
                    else:
                        act_copy(scalar, idx)

        nc.compile()
    return nc


def _get_program(scheme):
    if scheme not in _PROGRAM_CACHE:
        if scheme == "bf16s":
            _PROGRAM_CACHE[scheme] = _build_bf16s(
                nwarm=int(os.environ.get("BIATT_NWARM", "24"))
            )
        elif scheme == "raw":
            _PROGRAM_CACHE[scheme] = _build_raw()
        elif scheme == "bf16x2":
            _PROGRAM_CACHE[scheme] = _build_bf16x2()
        else:
            _PROGRAM_CACHE[scheme] = _build_f32(
                mybir.dt.float32r if scheme == "f32r" else _F32
            )
    return _PROGRAM_CACHE[scheme]


def _chunk_pieces(mat_t, dtype, npiece):
    """[K=512, len] -> npiece contiguous [128, 4/npiece, len] partition-major
    K-chunk groups."""
    ln = mat_t.shape[1]
    c = np.ascontiguousarray(
        mat_t.reshape(KC, P, ln).transpose(1, 0, 2).astype(dtype)
    )  # [128, 4, len]
    per = KC // npiece
    return [np.ascontiguousarray(c[:, i * per:(i + 1) * per]) for i in range(npiece)]


def _chunk_halves(mat_t, dtype):
    return _chunk_pieces(mat_t, dtype, 2)


def _split_hi_lo(a):
    hi = a.astype(ml_dtypes.bfloat16)
    lo = (a - hi.astype(np.float32)).astype(ml_dtypes.bfloat16)
    return hi, lo


def kernel(**inputs):
    global _LAST_EXEC_NS

    atoms = np.ascontiguousarray(np.asarray(inputs["atoms_vector"], dtype=np.float32))
    amino = np.ascontiguousarray(np.asarray(inputs["amino_vector"], dtype=np.float32))
    Wcc = np.asarray(inputs["Wcc"], dtype=np.float32)
    Wcp = np.asarray(inputs["Wcp"], dtype=np.float32)
    bcc = np.asarray(inputs["bcc"], dtype=np.float32)
    bcp = np.asarray(inputs["bcp"], dtype=np.float32)

    # Fold the four weight blocks (concat([v]*4, 1) @ W == v @ sum-of-blocks).
    wcc_f = Wcc.reshape(4, D, D).sum(axis=0)
    wcp_f = Wcp.reshape(4, D, D).sum(axis=0)

    scheme = os.environ.get("BIATT_MM", "bf16s")
    nc = _get_program(scheme)

    in_maps = []
    if scheme == "bf16s":
        # Stream-split sharding: cores 0-3 compute cf rows (atoms @ wcc_f),
        # cores 4-7 pf rows (amino @ wcp_f); 1536 rows per core.
        w_bf = {
            "cc": wcc_f.astype(ml_dtypes.bfloat16),
            "cp": wcp_f.astype(ml_dtypes.bfloat16),
        }
        for c in range(N_CORES):
            stream = "cc" if c < 4 else "cp"
            base = atoms if c < 4 else amino
            sl = slice((c % 4) * SHARD2, (c % 4 + 1) * SHARD2)
            xt = base[sl].T.astype(ml_dtypes.bfloat16)  # [512, 1536]
            m = {}
            wb = w_bf[stream]
            xb = np.empty((P, 2, 2, XH), dtype=ml_dtypes.bfloat16)
            for k in range(KC):
                chunk = xt[k * P:(k + 1) * P]
                if k == 0:
                    p0a = np.empty((P, D + P), dtype=ml_dtypes.bfloat16)
                    p0a[:, :D] = wb[:P]
                    p0a[:, D:] = chunk[:, :P]
                    m["p0a"] = p0a
                    m["p0b"] = np.ascontiguousarray(chunk[:, P:XH])
                else:
                    pk = np.empty((P, D + XH), dtype=ml_dtypes.bfloat16)
                    pk[:, :D] = wb[k * P:(k + 1) * P]
                    pk[:, D:] = chunk[:, :XH]
                    m[f"p{k}"] = pk
                xb[:, k // 2, k % 2] = chunk[:, XH:]
            m["xb01"] = np.ascontiguousarray(xb[:, 0])
            m["xb23"] = np.ascontiguousarray(xb[:, 1])
            in_maps.append(m)
    elif scheme in ("bf16x2", "raw"):
        # raw: wcch/xh in four per-chunk pieces, the rest in two halves;
        # tile bf16x2: everything in two halves.
        n_first = 2
        wcch, wccl = _split_hi_lo(wcc_f)
        wcph, wcpl = _split_hi_lo(wcp_f)
        w_parts = {}
        for nm, arr, npiece in (("wcch", wcch, n_first), ("wccl", wccl, 2),
                                ("wcph", wcph, 2), ("wcpl", wcpl, 2)):
            for i, p in enumerate(_chunk_pieces(arr, ml_dtypes.bfloat16, npiece)):
                w_parts[f"{nm}{i}"] = p
        for c in range(N_CORES):
            sl = slice(c * SHARD, (c + 1) * SHARD)
            m = dict(w_parts)
            for nm, base in (("x", atoms), ("y", amino)):
                t = base[sl].T  # [512, 768]
                hi, lo = _split_hi_lo(t)
                nh = n_first if nm == "x" else 2
                for i, p in enumerate(_chunk_pieces(hi, ml_dtypes.bfloat16, nh)):
                    m[f"{nm}h{i}"] = p
                for i, p in enumerate(_chunk_pieces(lo, ml_dtypes.bfloat16, 2)):
                    m[f"{nm}l{i}"] = p
            in_maps.append(m)
    else:
        w_parts = {}
        for nm, arr in (("wcc", wcc_f), ("wcp", wcp_f)):
            w_parts[f"{nm}0"], w_parts[f"{nm}1"] = _chunk_halves(arr, np.float32)
        for c in range(N_CORES):
            sl = slice(c * SHARD, (c + 1) * SHARD)
            m = dict(w_parts)
            m["x0"], m["x1"] = _chunk_halves(atoms[sl].T, np.float32)
            m["y0"], m["y1"] = _chunk_halves(amino[sl].T, np.float32)
            in_maps.append(m)

    trace = bool(os.environ.get("BIATT_TRACE"))
    try:
        res = run_bass_kernel_spmd(nc, in_maps, list(range(N_CORES)), trace=trace)
    except Exception:
        # One retry: a transiently wedged NeuronCore surfaces as a runtime
        # error on an otherwise-valid program.
        res = run_bass_kernel_spmd(nc, in_maps, list(range(N_CORES)), trace=trace)
    _LAST_EXEC_NS = res.exec_time_ns

    if scheme == "bf16s":
        def _unpack(c):
            # Device layout [128, 12, 512] (partition-major) -> [1536, 512].
            o = res.results[c]["out"]
            return o.transpose(1, 0, 2).reshape(SHARD2, D).astype(np.float32)

        cf = np.concatenate([_unpack(c) for c in range(4)], axis=0)
        pf = np.concatenate([_unpack(c) for c in range(4, 8)], axis=0)
    else:
        cf = np.concatenate(
            [res.results[c]["cf"].reshape(SHARD, D) for c in range(N_CORES)],
            axis=0,
        )
        pf = np.concatenate(
            [res.results[c]["pf"].reshape(SHARD, D) for c in range(N_CORES)],
            axis=0,
        )
    cf += bcc  # rank-1 epilogue on the gathered output
    pf += bcp
    return cf, pf



# revision 40
# speedup vs baseline: 2.9395x; 1.0028x over previous
"""BiATT kernel for 8 Trainium2 NeuronCores.

The reference module's bilinear-attention branch is dead code: the
"attention" weights are softmax(axis=1) over [N, 1] tensors, which is
exactly 1.0 for every row.  Hence

    cf_final = atoms_vector @ (Wcc[0:D] + Wcc[D:2D] + Wcc[2D:3D] + Wcc[3D:4D]) + bcc
    pf_final = amino_vector @ (Wcp[0:D] + Wcp[D:2D] + Wcp[2D:3D] + Wcp[3D:4D]) + bcp

bit-for-bit up to fp32 rounding, i.e. two [6144,512] @ [512,512] matmuls
with host-folded weights.

Default scheme "bf16s": stream-split sharding (cores 0-3 compute cf rows,
cores 4-7 pf rows, 1536 rows per core) with a SINGLE bf16 matmul term and
bf16 outputs, converted to f32 on the host.  Measured end-to-end error is
~3.5e-3 against the fp32 reference (the harness gate is 2e-2); per-core
traffic is 2.05 MB in + 1.57 MB out and 48 matmuls (805 MFLOP) -- one
third of the PE work and a quarter of the DMA bytes of the bf16x2 path.
See _build_bf16s for the pipeline schedule.  BIATT_MM selects the legacy
schemes (raw = hand-scheduled bf16x2, bf16x2 = Tile bf16x2, f32, f32r);
BIATT_NWARM tunes the warm-up burst.

The bias is added on the host during the gather (a rank-1 epilogue on the
full output).
"""

import os

import ml_dtypes
import numpy as np

import concourse.bacc as bacc
import concourse.bass as bass
import concourse.mybir as mybir
import concourse.tile as tile
from concourse.bass_utils import run_bass_kernel_spmd

N_CORES = 8
D = 512          # feature dim
N_ROWS = 6144    # rows of atoms_vector / amino_vector
SHARD = N_ROWS // N_CORES   # 768 rows per core
P = 128          # SBUF partitions
KC = D // P      # 4 contraction chunks
NRB = SHARD // P  # 6 row blocks per shard

_F32 = mybir.dt.float32
_BF16 = mybir.dt.bfloat16
_PROGRAM_CACHE = {}

_LAST_EXEC_NS = None


def _new_bass():
    return bacc.Bacc(
        "TRN2",
        target_bir_lowering=False,
        debug=False,
        num_devices=N_CORES,
    )


def _build_bf16x2():
    """Split-bf16 path: per stream (cc / cp) the activation comes as hi/lo
    bf16 halves and the folded weight as hi/lo bf16 halves.  Input tensors
    are partition-major K-chunked ([128, nk, len]) so each is one large
    contiguous DMA.  psum[rb] accumulates 12 matmuls: k0..3 of xh@wh,
    xl@wh, xh@wl.

    Perf structure: inputs are two-chunk halves loaded in consumption order
    on the Sync HWDGE ring (output DMAs ride the Activation ring so the two
    dispatch streams never serialize against each other); a burst of
    throwaway matmuls on scratch tiles keeps the PE busy during the DMA
    lead so the HAM clock gate is released (2.4 GHz) when the real matmul
    stream starts."""
    nc = _new_bass()

    # names: {tensor}{piece}; each tensor comes as 2 two-chunk halves.
    d = {}
    layout = {}
    for t, ln, npiece, nk in (
        ("xh", SHARD, 2, 2), ("wcch", D, 2, 2),
        ("xl", SHARD, 2, 2), ("wccl", D, 2, 2),
        ("yh", SHARD, 2, 2), ("wcph", D, 2, 2),
        ("yl", SHARD, 2, 2), ("wcpl", D, 2, 2),
    ):
        layout[t] = (ln, npiece, nk)
        for h in range(npiece):
            d[f"{t}{h}"] = nc.dram_tensor(
                f"{t}{h}", [P, nk, ln], _BF16, kind="ExternalInput"
            ).ap()

    cf = nc.dram_tensor("cf", [NRB, P, D], _F32, kind="ExternalOutput").ap()
    pf = nc.dram_tensor("pf", [NRB, P, D], _F32, kind="ExternalOutput").ap()

    with tile.TileContext(nc) as tc:
        with (
            tc.tile_pool(name="ins", bufs=1) as ins,
            tc.tile_pool(name="warm", bufs=1) as warm,
            tc.tile_pool(name="psum", bufs=7, space=bass.MemorySpace.PSUM) as psum,
            tc.tile_pool(name="wpsum", bufs=1, space=bass.MemorySpace.PSUM) as wpsum,
            tc.tile_pool(name="outs", bufs=8) as outs,
        ):
            # PE warm-up: ~4us of dependency-free matmuls on scratch data,
            # issued while the input DMAs stream in.  Keeps the HAM activity
            # window busy so the real matmuls run at 2.4 GHz from the start.
            wsrc = warm.tile([P, 2 * P], _BF16, tag="wsrc")
            nc.gpsimd.memset(wsrc[:], 0.0)
            wps = wpsum.tile([P, P], _F32, tag="wps")
            for i in range(40):
                nc.tensor.matmul(
                    wps[:], wsrc[:, 0:P], wsrc[:, P:2 * P],
                    start=(i == 0), stop=(i == 39),
                )

            # Load order == consumption order (cf stream first).
            s = {}
            def load(engine, name):
                ln, npiece, nk = layout[name[:-1]]
                t = ins.tile([P, nk, ln], _BF16, tag=name)
                engine.dma_start(t[:], d[name][:])
                s[name] = t

            for name in ("wcch0", "xh0", "wcch1", "xh1",
                         "xl0", "xl1", "wccl0", "wccl1",
                         "wcph0", "yh0", "wcph1", "yh1",
                         "yl0", "yl1", "wcpl0", "wcpl1"):
                load(nc.sync, name)

            def piece(t, k):
                ln, npiece, nk = layout[t]
                return s[f"{t}{k // nk}"][:, k % nk, :]

            for a, w, out_d in (("x", "wcc", cf), ("y", "wcp", pf)):
                for rb in range(NRB):
                    ps = psum.tile([P, D], _F32, tag="ps")
                    idx = 0
                    for ah, wh2 in ((f"{a}h", f"{w}h"), (f"{a}l", f"{w}h"),
                                    (f"{a}h", f"{w}l")):
                        for k in range(KC):
                            nc.tensor.matmul(
                                ps[:],
                                piece(ah, k)[:, rb * P:(rb + 1) * P],
                                piece(wh2, k),
                                start=(idx == 0),
                                stop=(idx == 3 * KC - 1),
                            )
                            idx += 1
                    ot = outs.tile([P, D], _F32, tag="ot")
                    nc.vector.tensor_copy(ot[:], ps[:])
                    nc.scalar.dma_start(out_d[rb], ot[:])

    nc.compile()
    return nc


_IN_ORDER = ("wcch0", "xh0", "wcch1", "xh1", "xl0", "xl1", "wccl0", "wccl1",
             "wcph0", "yh0", "wcph1", "yh1", "yl0", "yl1", "wcpl0", "wcpl1")

# ---------------------------------------------------------------------------
# Single-term bf16 scheme ("bf16s"): one stream per core (cores 0-3 compute
# cf rows, 4-7 pf rows; 1536 rows each), single bf16 matmul term (measured
# end-to-end error ~3.4e-3 vs the 2e-2 gate), bf16 outputs converted to f32
# on the host.  Per-core traffic: 2.05 MB in + 1.57 MB out vs the bf16x2
# path's 8.4 MB; PE work 48 matmuls (805 MFLOP) vs 144.
# ---------------------------------------------------------------------------

SHARD2 = N_ROWS // 4      # 1536 rows per core (4 cores per stream)
NRB2 = SHARD2 // P        # 12 row blocks
XH = SHARD2 // 2          # 768 columns per xT half (row-blocks 0-5 / 6-11)


# Output DMA plan: (blocks, queue) per DMA.  Early big pieces go on the
# Activation queue; the late pieces spread across queues so their
# HWDGE/SWDGE dispatch chains overlap.  "pool" DMAs use the SWDGE path,
# which does not occupy the shared HWDGE generator at all.
OUT_PLAN = ((3, "act"), (3, "act"), (2, "act"), (2, "act"), (1, "sync"),
            (1, "sync"))
OUT_SPLIT = tuple(n for n, _ in OUT_PLAN)


def _build_bf16s(nwarm=30, out_plan=OUT_PLAN):
    """Hand-scheduled raw pipeline, one [1536,512]@[512,512] bf16 matmul.

    DMA dispatch is the scarce resource (each HWDGE dispatch occupies the
    shared generator ~630ns, and the first DMA's SEQ+HWDGE+DGE lead is
    ~1.9us), so inputs are 7 DMAs: p0a/p0b split the k0 chunk (weight
    [128,512] packed with activation row-blocks 0-2 / 3-5) so the first
    matmul gates on 224KB; p1..p3 pack weight chunk k with activation
    chunk k of row-blocks 0-5; xb01/xb23 carry the activation chunks of
    row-blocks 6-11 two-at-a-time.  Outputs: row-blocks 0-10 are
    f32->bf16 copied (alternating DVE/GPSIMD, whose 658/806ns per block
    matches the 426ns stop stagger) into 4 private SBUF slots and stored
    by 4 Activation-queue DMAs (3/3/3/2 blocks, partition-major
    [128,12,512] DRAM, host re-transposes); block 11 is DMA'd from its
    PSUM bank as f32 on the idle sync queue, skipping the copy.

    The warm-up matmuls run on uninitialized SBUF from t~0 (their PSUM
    bank is reset by a later start=True) purely to hold the PE p-state at
    full clock before the real stream starts.

    Groups 0-5 (banks 0-5): k0,k1 k-outer, then per-rb (k2,k3) so stops
    stagger 426ns; groups 6-11 (banks 6,7,0-3) mirror, bank 0-3 reuse
    gated on the first-half copies."""
    from contextlib import ExitStack

    nc = _new_bass()

    d = {}
    d["p0a"] = nc.dram_tensor("p0a", [P, D + P], _BF16, kind="ExternalInput").ap()
    d["p0b"] = nc.dram_tensor("p0b", [P, 5 * P], _BF16, kind="ExternalInput").ap()
    for k in range(1, KC - 1):
        d[f"p{k}"] = nc.dram_tensor(
            f"p{k}", [P, D + XH], _BF16, kind="ExternalInput").ap()
    d["p3a"] = nc.dram_tensor("p3a", [P, D + 3 * P], _BF16, kind="ExternalInput").ap()
    d["p3b"] = nc.dram_tensor("p3b", [P, 3 * P], _BF16, kind="ExternalInput").ap()
    d["xb01"] = nc.dram_tensor("xb01", [P, 2, XH], _BF16, kind="ExternalInput").ap()
    d["xb23"] = nc.dram_tensor("xb23", [P, 2, XH], _BF16, kind="ExternalInput").ap()
    out_d = nc.dram_tensor("out", [P, NRB2, D], _BF16, kind="ExternalOutput").ap()

    out_split = tuple(n for n, _ in out_plan)
    DMA_ORDER = ("p0a", "p0b", "p1", "p2", "p3a", "p3b", "xb01", "xb23")
    SHAPES = {"p0a": [P, D + P], "p0b": [P, 5 * P],
              "p1": [P, D + XH], "p2": [P, D + XH],
              "p3a": [P, D + 3 * P], "p3b": [P, 3 * P],
              "xb01": [P, 2, XH], "xb23": [P, 2, XH]}
    NJ = len(out_split)

    with ExitStack() as ctx:
        sb = {
            name: ctx.enter_context(
                nc.sbuf_tensor(f"sb_{name}", SHAPES[name], _BF16))
            for name in DMA_ORDER
        }
        # One private slot per output DMA -- no ping-pong waits anywhere.
        max_n = max(out_split)
        outsb = ctx.enter_context(
            nc.sbuf_tensor("outsb", [P, NJ, max_n * D], _BF16))
        warm = ctx.enter_context(nc.sbuf_tensor("warmsb", [P, 2 * P], _BF16))
        ps = [
            ctx.enter_context(nc.psum_tensor(f"psum{i}", [P, D], _F32))
            for i in range(8)
        ]
        s_mm = ctx.enter_context(nc.semaphore("s_mm"))
        s_cpv = ctx.enter_context(nc.semaphore("s_cpv"))  # DVE copies
        s_cpg = ctx.enter_context(nc.semaphore("s_cpg"))  # GPSIMD copies
        s_cpa = ctx.enter_context(nc.semaphore("s_cpa"))  # ACT copies
        s_od = ctx.enter_context(nc.semaphore("s_od"))    # out-DMA completions
        gates = {
            name: ctx.enter_context(nc.semaphore(f"s_{name}"))
            for name in DMA_ORDER
        }

        blk_dma = []
        for j, n in enumerate(out_split):
            for o in range(n):
                blk_dma.append((j, o))
        dma_first_blk = [sum(out_split[:j]) for j in range(NJ)]
        # 0 = DVE, 1 = GPSIMD, 2 = ACT (unused by default).
        cp_eng = [g % 2 for g in range(NRB2)]
        cp_sem_of = {0: s_cpv, 1: s_cpg, 2: s_cpa}

        def cp_counts(last_blk):
            return tuple(
                sum(1 for g in range(NRB2) if cp_eng[g] == e and g <= last_blk)
                for e in range(3)
            )

        def wchunk(k):
            if k == 0:
                return sb["p0a"][:, 0:D]
            if k == 3:
                return sb["p3a"][:, 0:D]
            return sb[f"p{k}"][:, 0:D]

        def xblk(half, k, rb):
            if half == "a":
                if k == 0:
                    t, r = ("p0a", rb) if rb < 1 else ("p0b", rb - 1)
                    off = D if t == "p0a" else 0
                    return sb[t][:, off + r * P:off + (r + 1) * P]
                if k == 3:
                    t, r = ("p3a", rb) if rb < 3 else ("p3b", rb - 3)
                    off = D if t == "p3a" else 0
                    return sb[t][:, off + r * P:off + (r + 1) * P]
                return sb[f"p{k}"][:, D + rb * P:D + (rb + 1) * P]
            return sb["xb01" if k < 2 else "xb23"][:, k % 2, rb * P:(rb + 1) * P]

        def copy_loop(eng_idx, engine, eng_ns, sem):
            for g in range(NRB2):
                if cp_eng[g] != eng_idx:
                    continue
                j, o = blk_dma[g]
                engine.wait_ge(s_mm, g + 1)
                eng_ns.tensor_copy(
                    outsb[:, j, o * D:(o + 1) * D], ps[g % 8][:]
                ).then_inc(sem, 1)

        with nc.Block() as block:

            def issue_out(handle, eng_ns, j):
                n = out_split[j]
                b0 = dma_first_blk[j]
                counts = cp_counts(b0 + n - 1)
                for e, cnt in enumerate(counts):
                    if cnt:
                        handle.wait_ge(cp_sem_of[e], cnt)
                eng_ns.dma_start(
                    out_d[:, b0:b0 + n, :], outsb[:, j, 0:n * D]
                ).then_inc(s_od, 16)

            def act_copy(scalar, g):
                j, o = blk_dma[g]
                scalar.wait_ge(s_mm, g + 1)
                nc.scalar.activation(
                    outsb[:, j, o * D:(o + 1) * D], ps[g % 8][:],
                    mybir.ActivationFunctionType.Copy,
                ).then_inc(s_cpa, 1)

            @block.sync
            def _(sync):
                for name in DMA_ORDER:
                    if name == "p0a":
                        continue  # issued on the Pool/SWDGE queue
                    sync.dma_start(sb[name][:], d[name][:]).then_inc(
                        gates[name], 16
                    )
                for j, (n, q) in enumerate(out_plan):
                    if q == "sync":
                        issue_out(sync, nc.sync, j)

            @block.vector
            def _(vector):
                copy_loop(0, vector, nc.vector, s_cpv)
                for j, (n, q) in enumerate(out_plan):
                    if q == "vec":
                        issue_out(vector, nc.vector, j)

            @block.gpsimd
            def _(gpsimd):
                nc.gpsimd.dma_start(sb["p0a"][:], d["p0a"][:]).then_inc(
                    gates["p0a"], 16
                )
                for j, (n, q) in enumerate(out_plan):
                    if q == "pool":
                        issue_out(gpsimd, nc.gpsimd, j)
                for j, (n, q) in enumerate(out_plan):
                    if q == "pool":
                        issue_out(gpsimd, nc.gpsimd, j)

            @block.tensor
            def _(tensor):
                for i in range(nwarm):
                    nc.tensor.matmul(
                        ps[7][:, 0:P], warm[:, 0:P], warm[:, P:2 * P],
                        start=(i == 0), stop=(i == nwarm - 1),
                    )
                waited = set()

                def gate(name):
                    if name not in waited:
                        waited.add(name)
                        tensor.wait_ge(gates[name], 16)

                def bank_wait(g):
                    if g >= 8:
                        e = cp_eng[g - 8]
                        tensor.wait_ge(cp_sem_of[e], cp_counts(g - 8)[e])

                def mm(half, k, rb, start, stop):
                    g = (0 if half == "a" else 6) + rb
                    if start:
                        bank_wait(g)
                    m = nc.tensor.matmul(
                        ps[g % 8][:], xblk(half, k, rb), wchunk(k),
                        start=start, stop=stop,
                    )
                    if stop:
                        m.then_inc(s_mm, 1)

                # Phase a: the first gate blocks the PE SEQ, and the ~5
                # instructions issued after any blocking wait run at the
                # mid p-state -- burn them on short throwaway matmuls so
                # every real matmul runs at full clock.
                gate("p0a")
                for i in range(5):
                    nc.tensor.matmul(
                        ps[7][:, 0:32], warm[:, 0:P], warm[:, P:P + 32],
                        start=(i == 0), stop=(i == 4),
                    )
                for rb in range(6):
                    if rb == 1:
                        gate("p0b")
                    mm("a", 0, rb, True, False)
                gate("p1")
                for rb in range(6):
                    mm("a", 1, rb, False, False)
                gate("p2")
                gate("p3a")
                for rb in range(6):
                    if rb == 3:
                        gate("p3b")
                    mm("a", 2, rb, False, False)
                    mm("a", 3, rb, False, True)
                # Phase b: k0 k-outer; per-rb (k1,k2,k3) so stops stagger
                # 639ns apart, matching the DVE copy throughput.
                gate("xb01")
                for rb in range(6):
                    mm("b", 0, rb, True, False)
                for rb in range(6):
                    mm("b", 1, rb, False, False)
                    if rb == 0:
                        gate("xb23")
                    mm("b", 2, rb, False, False)
                    mm("b", 3, rb, False, True)

            @block.scalar
            def _(scalar):
                # Dummy activation so the Copy act-table is loaded long
                # before the first real copy.
                nc.scalar.activation(
                    outsb[:, 0, 0:32], warm[:, 0:32],
                    mybir.ActivationFunctionType.Copy,
                )
                # Interleave ACT copies and this queue's out-DMAs in
                # dependency order: copy g keyed g, DMA j keyed by its last
                # block + 0.5, so every DMA follows the copies it waits on.
                items = []
                for j, (n, q) in enumerate(out_plan):
                    if q == "act":
                        items.append((dma_first_blk[j] + n - 0.5, "dma", j))
                for g in range(NRB2):
                    if cp_eng[g] == 2:
                        items.append((float(g), "cp", g))
                for _, kind, idx in sorted(items):
                    if kind == "dma":
                        issue_out(scalar, nc.scalar, idx)
                    else:
                        act_copy(scalar, idx)

        nc.compile()
    return nc


def _get_program(scheme):
    if scheme not in _PROGRAM_CACHE:
        if scheme == "bf16s":
            _PROGRAM_CACHE[scheme] = _build_bf16s(
                nwarm=int(os.environ.get("BIATT_NWARM", "24"))
            )
        elif scheme == "raw":
            _PROGRAM_CACHE[scheme] = _build_raw()
        elif scheme == "bf16x2":
            _PROGRAM_CACHE[scheme] = _build_bf16x2()
        else:
            _PROGRAM_CACHE[scheme] = _build_f32(
                mybir.dt.float32r if scheme == "f32r" else _F32
            )
    return _PROGRAM_CACHE[scheme]


def _chunk_pieces(mat_t, dtype, npiece):
    """[K=512, len] -> npiece contiguous [128, 4/npiece, len] partition-major
    K-chunk groups."""
    ln = mat_t.shape[1]
    c = np.ascontiguousarray(
        mat_t.reshape(KC, P, ln).transpose(1, 0, 2).astype(dtype)
    )  # [128, 4, len]
    per = KC // npiece
    return [np.ascontiguousarray(c[:, i * per:(i + 1) * per]) for i in range(npiece)]


def _chunk_halves(mat_t, dtype):
    return _chunk_pieces(mat_t, dtype, 2)


def _split_hi_lo(a):
    hi = a.astype(ml_dtypes.bfloat16)
    lo = (a - hi.astype(np.float32)).astype(ml_dtypes.bfloat16)
    return hi, lo


def kernel(**inputs):
    global _LAST_EXEC_NS

    atoms = np.ascontiguousarray(np.asarray(inputs["atoms_vector"], dtype=np.float32))
    amino = np.ascontiguousarray(np.asarray(inputs["amino_vector"], dtype=np.float32))
    Wcc = np.asarray(inputs["Wcc"], dtype=np.float32)
    Wcp = np.asarray(inputs["Wcp"], dtype=np.float32)
    bcc = np.asarray(inputs["bcc"], dtype=np.float32)
    bcp = np.asarray(inputs["bcp"], dtype=np.float32)

    # Fold the four weight blocks (concat([v]*4, 1) @ W == v @ sum-of-blocks).
    wcc_f = Wcc.reshape(4, D, D).sum(axis=0)
    wcp_f = Wcp.reshape(4, D, D).sum(axis=0)

    scheme = os.environ.get("BIATT_MM", "bf16s")
    nc = _get_program(scheme)

    in_maps = []
    if scheme == "bf16s":
        # Stream-split sharding: cores 0-3 compute cf rows (atoms @ wcc_f),
        # cores 4-7 pf rows (amino @ wcp_f); 1536 rows per core.
        w_bf = {
            "cc": wcc_f.astype(ml_dtypes.bfloat16),
            "cp": wcp_f.astype(ml_dtypes.bfloat16),
        }
        for c in range(N_CORES):
            stream = "cc" if c < 4 else "cp"
            base = atoms if c < 4 else amino
            sl = slice((c % 4) * SHARD2, (c % 4 + 1) * SHARD2)
            xt = base[sl].T.astype(ml_dtypes.bfloat16)  # [512, 1536]
            m = {}
            wb = w_bf[stream]
            xb = np.empty((P, 2, 2, XH), dtype=ml_dtypes.bfloat16)
            for k in range(KC):
                chunk = xt[k * P:(k + 1) * P]
                if k == 0:
                    p0a = np.empty((P, D + P), dtype=ml_dtypes.bfloat16)
                    p0a[:, :D] = wb[:P]
                    p0a[:, D:] = chunk[:, :P]
                    m["p0a"] = p0a
                    m["p0b"] = np.ascontiguousarray(chunk[:, P:XH])
                elif k == 3:
                    p3a = np.empty((P, D + 3 * P), dtype=ml_dtypes.bfloat16)
                    p3a[:, :D] = wb[k * P:(k + 1) * P]
                    p3a[:, D:] = chunk[:, :3 * P]
                    m["p3a"] = p3a
                    m["p3b"] = np.ascontiguousarray(chunk[:, 3 * P:XH])
                else:
                    pk = np.empty((P, D + XH), dtype=ml_dtypes.bfloat16)
                    pk[:, :D] = wb[k * P:(k + 1) * P]
                    pk[:, D:] = chunk[:, :XH]
                    m[f"p{k}"] = pk
                xb[:, k // 2, k % 2] = chunk[:, XH:]
            m["xb01"] = np.ascontiguousarray(xb[:, 0])
            m["xb23"] = np.ascontiguousarray(xb[:, 1])
            in_maps.append(m)
    elif scheme in ("bf16x2", "raw"):
        # raw: wcch/xh in four per-chunk pieces, the rest in two halves;
        # tile bf16x2: everything in two halves.
        n_first = 2
        wcch, wccl = _split_hi_lo(wcc_f)
        wcph, wcpl = _split_hi_lo(wcp_f)
        w_parts = {}
        for nm, arr, npiece in (("wcch", wcch, n_first), ("wccl", wccl, 2),
                                ("wcph", wcph, 2), ("wcpl", wcpl, 2)):
            for i, p in enumerate(_chunk_pieces(arr, ml_dtypes.bfloat16, npiece)):
                w_parts[f"{nm}{i}"] = p
        for c in range(N_CORES):
            sl = slice(c * SHARD, (c + 1) * SHARD)
            m = dict(w_parts)
            for nm, base in (("x", atoms), ("y", amino)):
                t = base[sl].T  # [512, 768]
                hi, lo = _split_hi_lo(t)
                nh = n_first if nm == "x" else 2
                for i, p in enumerate(_chunk_pieces(hi, ml_dtypes.bfloat16, nh)):
                    m[f"{nm}h{i}"] = p
                for i, p in enumerate(_chunk_pieces(lo, ml_dtypes.bfloat16, 2)):
                    m[f"{nm}l{i}"] = p
            in_maps.append(m)
    else:
        w_parts = {}
        for nm, arr in (("wcc", wcc_f), ("wcp", wcp_f)):
            w_parts[f"{nm}0"], w_parts[f"{nm}1"] = _chunk_halves(arr, np.float32)
        for c in range(N_CORES):
            sl = slice(c * SHARD, (c + 1) * SHARD)
            m = dict(w_parts)
            m["x0"], m["x1"] = _chunk_halves(atoms[sl].T, np.float32)
            m["y0"], m["y1"] = _chunk_halves(amino[sl].T, np.float32)
            in_maps.append(m)

    trace = bool(os.environ.get("BIATT_TRACE"))
    try:
        res = run_bass_kernel_spmd(nc, in_maps, list(range(N_CORES)), trace=trace)
    except Exception:
        # One retry: a transiently wedged NeuronCore surfaces as a runtime
        # error on an otherwise-valid program.
        res = run_bass_kernel_spmd(nc, in_maps, list(range(N_CORES)), trace=trace)
    _LAST_EXEC_NS = res.exec_time_ns

    if scheme == "bf16s":
        def _unpack(c):
            # Device layout [128, 12, 512] (partition-major) -> [1536, 512].
            o = res.results[c]["out"]
            return o.transpose(1, 0, 2).reshape(SHARD2, D).astype(np.float32)

        cf = np.concatenate([_unpack(c) for c in range(4)], axis=0)
        pf = np.concatenate([_unpack(c) for c in range(4, 8)], axis=0)
    else:
        cf = np.concatenate(
            [res.results[c]["cf"].reshape(SHARD, D) for c in range(N_CORES)],
            axis=0,
        )
        pf = np.concatenate(
            [res.results[c]["pf"].reshape(SHARD, D) for c in range(N_CORES)],
            axis=0,
        )
    cf += bcc  # rank-1 epilogue on the gathered output
    pf += bcp
    return cf, pf



# revision 42
# speedup vs baseline: 2.9773x; 1.0129x over previous
"""BiATT kernel for 8 Trainium2 NeuronCores.

The reference module's bilinear-attention branch is dead code: the
"attention" weights are softmax(axis=1) over [N, 1] tensors, which is
exactly 1.0 for every row.  Hence

    cf_final = atoms_vector @ (Wcc[0:D] + Wcc[D:2D] + Wcc[2D:3D] + Wcc[3D:4D]) + bcc
    pf_final = amino_vector @ (Wcp[0:D] + Wcp[D:2D] + Wcp[2D:3D] + Wcp[3D:4D]) + bcp

bit-for-bit up to fp32 rounding, i.e. two [6144,512] @ [512,512] matmuls
with host-folded weights.

Default scheme "bf16s": stream-split sharding (cores 0-3 compute cf rows,
cores 4-7 pf rows, 1536 rows per core) with a SINGLE bf16 matmul term and
bf16 outputs, converted to f32 on the host.  Measured end-to-end error is
~3.5e-3 against the fp32 reference (the harness gate is 2e-2); per-core
traffic is 2.05 MB in + 1.57 MB out and 48 matmuls (805 MFLOP) -- one
third of the PE work and a quarter of the DMA bytes of the bf16x2 path.
See _build_bf16s for the pipeline schedule.  BIATT_MM selects the legacy
schemes (raw = hand-scheduled bf16x2, bf16x2 = Tile bf16x2, f32, f32r);
BIATT_NWARM tunes the warm-up burst.

The bias is added on the host during the gather (a rank-1 epilogue on the
full output).
"""

import os

import ml_dtypes
import numpy as np

import concourse.bacc as bacc
import concourse.bass as bass
import concourse.mybir as mybir
import concourse.tile as tile
from concourse.bass_utils import run_bass_kernel_spmd

N_CORES = 8
D = 512          # feature dim
N_ROWS = 6144    # rows of atoms_vector / amino_vector
SHARD = N_ROWS // N_CORES   # 768 rows per core
P = 128          # SBUF partitions
KC = D // P      # 4 contraction chunks
NRB = SHARD // P  # 6 row blocks per shard

_F32 = mybir.dt.float32
_BF16 = mybir.dt.bfloat16
_PROGRAM_CACHE = {}

_LAST_EXEC_NS = None


def _new_bass():
    return bacc.Bacc(
        "TRN2",
        target_bir_lowering=False,
        debug=False,
        num_devices=N_CORES,
    )


def _build_bf16x2():
    """Split-bf16 path: per stream (cc / cp) the activation comes as hi/lo
    bf16 halves and the folded weight as hi/lo bf16 halves.  Input tensors
    are partition-major K-chunked ([128, nk, len]) so each is one large
    contiguous DMA.  psum[rb] accumulates 12 matmuls: k0..3 of xh@wh,
    xl@wh, xh@wl.

    Perf structure: inputs are two-chunk halves loaded in consumption order
    on the Sync HWDGE ring (output DMAs ride the Activation ring so the two
    dispatch streams never serialize against each other); a burst of
    throwaway matmuls on scratch tiles keeps the PE busy during the DMA
    lead so the HAM clock gate is released (2.4 GHz) when the real matmul
    stream starts."""
    nc = _new_bass()

    # names: {tensor}{piece}; each tensor comes as 2 two-chunk halves.
    d = {}
    layout = {}
    for t, ln, npiece, nk in (
        ("xh", SHARD, 2, 2), ("wcch", D, 2, 2),
        ("xl", SHARD, 2, 2), ("wccl", D, 2, 2),
        ("yh", SHARD, 2, 2), ("wcph", D, 2, 2),
        ("yl", SHARD, 2, 2), ("wcpl", D, 2, 2),
    ):
        layout[t] = (ln, npiece, nk)
        for h in range(npiece):
            d[f"{t}{h}"] = nc.dram_tensor(
                f"{t}{h}", [P, nk, ln], _BF16, kind="ExternalInput"
            ).ap()

    cf = nc.dram_tensor("cf", [NRB, P, D], _F32, kind="ExternalOutput").ap()
    pf = nc.dram_tensor("pf", [NRB, P, D], _F32, kind="ExternalOutput").ap()

    with tile.TileContext(nc) as tc:
        with (
            tc.tile_pool(name="ins", bufs=1) as ins,
            tc.tile_pool(name="warm", bufs=1) as warm,
            tc.tile_pool(name="psum", bufs=7, space=bass.MemorySpace.PSUM) as psum,
            tc.tile_pool(name="wpsum", bufs=1, space=bass.MemorySpace.PSUM) as wpsum,
            tc.tile_pool(name="outs", bufs=8) as outs,
        ):
            # PE warm-up: ~4us of dependency-free matmuls on scratch data,
            # issued while the input DMAs stream in.  Keeps the HAM activity
            # window busy so the real matmuls run at 2.4 GHz from the start.
            wsrc = warm.tile([P, 2 * P], _BF16, tag="wsrc")
            nc.gpsimd.memset(wsrc[:], 0.0)
            wps = wpsum.tile([P, P], _F32, tag="wps")
            for i in range(40):
                nc.tensor.matmul(
                    wps[:], wsrc[:, 0:P], wsrc[:, P:2 * P],
                    start=(i == 0), stop=(i == 39),
                )

            # Load order == consumption order (cf stream first).
            s = {}
            def load(engine, name):
                ln, npiece, nk = layout[name[:-1]]
                t = ins.tile([P, nk, ln], _BF16, tag=name)
                engine.dma_start(t[:], d[name][:])
                s[name] = t

            for name in ("wcch0", "xh0", "wcch1", "xh1",
                         "xl0", "xl1", "wccl0", "wccl1",
                         "wcph0", "yh0", "wcph1", "yh1",
                         "yl0", "yl1", "wcpl0", "wcpl1"):
                load(nc.sync, name)

            def piece(t, k):
                ln, npiece, nk = layout[t]
                return s[f"{t}{k // nk}"][:, k % nk, :]

            for a, w, out_d in (("x", "wcc", cf), ("y", "wcp", pf)):
                for rb in range(NRB):
                    ps = psum.tile([P, D], _F32, tag="ps")
                    idx = 0
                    for ah, wh2 in ((f"{a}h", f"{w}h"), (f"{a}l", f"{w}h"),
                                    (f"{a}h", f"{w}l")):
                        for k in range(KC):
                            nc.tensor.matmul(
                                ps[:],
                                piece(ah, k)[:, rb * P:(rb + 1) * P],
                                piece(wh2, k),
                                start=(idx == 0),
                                stop=(idx == 3 * KC - 1),
                            )
                            idx += 1
                    ot = outs.tile([P, D], _F32, tag="ot")
                    nc.vector.tensor_copy(ot[:], ps[:])
                    nc.scalar.dma_start(out_d[rb], ot[:])

    nc.compile()
    return nc


_IN_ORDER = ("wcch0", "xh0", "wcch1", "xh1", "xl0", "xl1", "wccl0", "wccl1",
             "wcph0", "yh0", "wcph1", "yh1", "yl0", "yl1", "wcpl0", "wcpl1")

# ---------------------------------------------------------------------------
# Single-term bf16 scheme ("bf16s"): one stream per core (cores 0-3 compute
# cf rows, 4-7 pf rows; 1536 rows each), single bf16 matmul term (measured
# end-to-end error ~3.4e-3 vs the 2e-2 gate), bf16 outputs converted to f32
# on the host.  Per-core traffic: 2.05 MB in + 1.57 MB out vs the bf16x2
# path's 8.4 MB; PE work 48 matmuls (805 MFLOP) vs 144.
# ---------------------------------------------------------------------------

SHARD2 = N_ROWS // 4      # 1536 rows per core (4 cores per stream)
NRB2 = SHARD2 // P        # 12 row blocks
XH = SHARD2 // 2          # 768 columns per xT half (row-blocks 0-5 / 6-11)


# Output DMA plan: (blocks, queue) per DMA.  Early big pieces go on the
# Activation queue; the late pieces spread across queues so their
# HWDGE/SWDGE dispatch chains overlap.  "pool" DMAs use the SWDGE path,
# which does not occupy the shared HWDGE generator at all.
OUT_PLAN = ((3, "act"), (3, "act"), (2, "act"), (2, "act"), (1, "sync"),
            (1, "sync"))
OUT_SPLIT = tuple(n for n, _ in OUT_PLAN)


def _build_bf16s(nwarm=30, out_plan=OUT_PLAN):
    """Hand-scheduled raw pipeline, one [1536,512]@[512,512] bf16 matmul.

    DMA dispatch is the scarce resource (each HWDGE dispatch occupies the
    shared generator ~630ns, and the first DMA's SEQ+HWDGE+DGE lead is
    ~1.9us), so inputs are 7 DMAs: p0a/p0b split the k0 chunk (weight
    [128,512] packed with activation row-blocks 0-2 / 3-5) so the first
    matmul gates on 224KB; p1..p3 pack weight chunk k with activation
    chunk k of row-blocks 0-5; xb01/xb23 carry the activation chunks of
    row-blocks 6-11 two-at-a-time.  Outputs: row-blocks 0-10 are
    f32->bf16 copied (alternating DVE/GPSIMD, whose 658/806ns per block
    matches the 426ns stop stagger) into 4 private SBUF slots and stored
    by 4 Activation-queue DMAs (3/3/3/2 blocks, partition-major
    [128,12,512] DRAM, host re-transposes); block 11 is DMA'd from its
    PSUM bank as f32 on the idle sync queue, skipping the copy.

    The warm-up matmuls run on uninitialized SBUF from t~0 (their PSUM
    bank is reset by a later start=True) purely to hold the PE p-state at
    full clock before the real stream starts.

    Groups 0-5 (banks 0-5): k0,k1 k-outer, then per-rb (k2,k3) so stops
    stagger 426ns; groups 6-11 (banks 6,7,0-3) mirror, bank 0-3 reuse
    gated on the first-half copies."""
    from contextlib import ExitStack

    nc = _new_bass()

    d = {}
    d["p0a"] = nc.dram_tensor("p0a", [P, D + 2 * P], _BF16, kind="ExternalInput").ap()
    d["p0b"] = nc.dram_tensor("p0b", [P, 4 * P], _BF16, kind="ExternalInput").ap()
    for k in range(1, KC - 1):
        d[f"p{k}"] = nc.dram_tensor(
            f"p{k}", [P, D + XH], _BF16, kind="ExternalInput").ap()
    d["p3a"] = nc.dram_tensor("p3a", [P, D + 3 * P], _BF16, kind="ExternalInput").ap()
    d["p3b"] = nc.dram_tensor("p3b", [P, 3 * P], _BF16, kind="ExternalInput").ap()
    d["xb01"] = nc.dram_tensor("xb01", [P, 2, XH], _BF16, kind="ExternalInput").ap()
    d["xb23"] = nc.dram_tensor("xb23", [P, 2, XH], _BF16, kind="ExternalInput").ap()
    out_d = nc.dram_tensor("out", [P, NRB2, D], _BF16, kind="ExternalOutput").ap()

    out_split = tuple(n for n, _ in out_plan)
    DMA_ORDER = ("p0a", "p0b", "p1", "p2", "p3a", "p3b", "xb01", "xb23")
    SHAPES = {"p0a": [P, D + 2 * P], "p0b": [P, 4 * P],
              "p1": [P, D + XH], "p2": [P, D + XH],
              "p3a": [P, D + 3 * P], "p3b": [P, 3 * P],
              "xb01": [P, 2, XH], "xb23": [P, 2, XH]}
    NJ = len(out_split)

    with ExitStack() as ctx:
        sb = {
            name: ctx.enter_context(
                nc.sbuf_tensor(f"sb_{name}", SHAPES[name], _BF16))
            for name in DMA_ORDER
        }
        # One private slot per output DMA -- no ping-pong waits anywhere.
        max_n = max(out_split)
        outsb = ctx.enter_context(
            nc.sbuf_tensor("outsb", [P, NJ, max_n * D], _BF16))
        warm = ctx.enter_context(nc.sbuf_tensor("warmsb", [P, 2 * P], _BF16))
        ps = [
            ctx.enter_context(nc.psum_tensor(f"psum{i}", [P, D], _F32))
            for i in range(8)
        ]
        s_mm = ctx.enter_context(nc.semaphore("s_mm"))
        s_cpv = ctx.enter_context(nc.semaphore("s_cpv"))  # DVE copies
        s_cpg = ctx.enter_context(nc.semaphore("s_cpg"))  # GPSIMD copies
        s_cpa = ctx.enter_context(nc.semaphore("s_cpa"))  # ACT copies
        s_od = ctx.enter_context(nc.semaphore("s_od"))    # out-DMA completions
        gates = {
            name: ctx.enter_context(nc.semaphore(f"s_{name}"))
            for name in DMA_ORDER
        }

        blk_dma = []
        for j, n in enumerate(out_split):
            for o in range(n):
                blk_dma.append((j, o))
        dma_first_blk = [sum(out_split[:j]) for j in range(NJ)]
        # 0 = DVE, 1 = GPSIMD, 2 = ACT (unused by default).
        cp_eng = [g % 2 for g in range(NRB2)]
        cp_sem_of = {0: s_cpv, 1: s_cpg, 2: s_cpa}

        def cp_counts(last_blk):
            return tuple(
                sum(1 for g in range(NRB2) if cp_eng[g] == e and g <= last_blk)
                for e in range(3)
            )

        def wchunk(k):
            if k == 0:
                return sb["p0a"][:, 0:D]
            if k == 3:
                return sb["p3a"][:, 0:D]
            return sb[f"p{k}"][:, 0:D]

        def xblk(half, k, rb):
            if half == "a":
                if k == 0:
                    t, r = ("p0a", rb) if rb < 2 else ("p0b", rb - 2)
                    off = D if t == "p0a" else 0
                    return sb[t][:, off + r * P:off + (r + 1) * P]
                if k == 3:
                    t, r = ("p3a", rb) if rb < 3 else ("p3b", rb - 3)
                    off = D if t == "p3a" else 0
                    return sb[t][:, off + r * P:off + (r + 1) * P]
                return sb[f"p{k}"][:, D + rb * P:D + (rb + 1) * P]
            return sb["xb01" if k < 2 else "xb23"][:, k % 2, rb * P:(rb + 1) * P]

        def copy_loop(eng_idx, engine, eng_ns, sem):
            for g in range(NRB2):
                if cp_eng[g] != eng_idx:
                    continue
                j, o = blk_dma[g]
                engine.wait_ge(s_mm, g + 1)
                eng_ns.tensor_copy(
                    outsb[:, j, o * D:(o + 1) * D], ps[g % 8][:]
                ).then_inc(sem, 1)

        with nc.Block() as block:

            def issue_out(handle, eng_ns, j):
                n = out_split[j]
                b0 = dma_first_blk[j]
                counts = cp_counts(b0 + n - 1)
                for e, cnt in enumerate(counts):
                    if cnt:
                        handle.wait_ge(cp_sem_of[e], cnt)
                eng_ns.dma_start(
                    out_d[:, b0:b0 + n, :], outsb[:, j, 0:n * D]
                ).then_inc(s_od, 16)

            def act_copy(scalar, g):
                j, o = blk_dma[g]
                scalar.wait_ge(s_mm, g + 1)
                nc.scalar.activation(
                    outsb[:, j, o * D:(o + 1) * D], ps[g % 8][:],
                    mybir.ActivationFunctionType.Copy,
                ).then_inc(s_cpa, 1)

            @block.sync
            def _(sync):
                for name in DMA_ORDER:
                    if name in ("p0b", "xb23"):
                        continue  # issued on the Pool/SWDGE queue
                    sync.dma_start(sb[name][:], d[name][:]).then_inc(
                        gates[name], 16
                    )
                for j, (n, q) in enumerate(out_plan):
                    if q == "sync":
                        issue_out(sync, nc.sync, j)

            @block.vector
            def _(vector):
                copy_loop(0, vector, nc.vector, s_cpv)
                for j, (n, q) in enumerate(out_plan):
                    if q == "vec":
                        issue_out(vector, nc.vector, j)

            @block.gpsimd
            def _(gpsimd):
                # p0b rides SWDGE (no shared-HWDGE occupancy) so the sync
                # chain's HWDGE slots all go to the k1..k3 gate pieces;
                # xb23 is consumed last and dispatches only after p1 lands
                # so its transfer does not displace the earlier gates.
                nc.gpsimd.dma_start(sb["p0b"][:], d["p0b"][:]).then_inc(
                    gates["p0b"], 16
                )
                gpsimd.wait_ge(gates["p1"], 16)
                nc.gpsimd.dma_start(sb["xb23"][:], d["xb23"][:]).then_inc(
                    gates["xb23"], 16
                )
                for j, (n, q) in enumerate(out_plan):
                    if q == "pool":
                        issue_out(gpsimd, nc.gpsimd, j)
                for j, (n, q) in enumerate(out_plan):
                    if q == "pool":
                        issue_out(gpsimd, nc.gpsimd, j)

            @block.tensor
            def _(tensor):
                for i in range(nwarm):
                    nc.tensor.matmul(
                        ps[7][:, 0:P], warm[:, 0:P], warm[:, P:2 * P],
                        start=(i == 0), stop=(i == nwarm - 1),
                    )
                waited = set()

                def gate(name):
                    if name not in waited:
                        waited.add(name)
                        tensor.wait_ge(gates[name], 16)

                def bank_wait(g):
                    if g >= 8:
                        e = cp_eng[g - 8]
                        tensor.wait_ge(cp_sem_of[e], cp_counts(g - 8)[e])

                def mm(half, k, rb, start, stop):
                    g = (0 if half == "a" else 6) + rb
                    if start:
                        bank_wait(g)
                    m = nc.tensor.matmul(
                        ps[g % 8][:], xblk(half, k, rb), wchunk(k),
                        start=start, stop=stop,
                    )
                    if stop:
                        m.then_inc(s_mm, 1)

                # Phase a: the first gate blocks the PE SEQ, and the ~5
                # instructions issued after any blocking wait run at the
                # mid p-state -- burn them on short throwaway matmuls so
                # every real matmul runs at full clock.
                gate("p0a")
                for i in range(5):
                    nc.tensor.matmul(
                        ps[7][:, 0:32], warm[:, 0:P], warm[:, P:P + 32],
                        start=(i == 0), stop=(i == 4),
                    )
                for rb in range(6):
                    if rb == 2:
                        gate("p0b")
                    mm("a", 0, rb, True, False)
                gate("p1")
                for rb in range(6):
                    mm("a", 1, rb, False, False)
                gate("p2")
                gate("p3a")
                for rb in range(6):
                    if rb == 3:
                        gate("p3b")
                    mm("a", 2, rb, False, False)
                    mm("a", 3, rb, False, True)
                # Phase b: k0 k-outer; per-rb (k1,k2,k3) so stops stagger
                # 639ns apart, matching the DVE copy throughput.
                gate("xb01")
                for rb in range(6):
                    mm("b", 0, rb, True, False)
                for rb in range(6):
                    mm("b", 1, rb, False, False)
                    if rb == 0:
                        gate("xb23")
                    mm("b", 2, rb, False, False)
                    mm("b", 3, rb, False, True)

            @block.scalar
            def _(scalar):
                # Dummy activation so the Copy act-table is loaded long
                # before the first real copy.
                nc.scalar.activation(
                    outsb[:, 0, 0:32], warm[:, 0:32],
                    mybir.ActivationFunctionType.Copy,
                )
                # Interleave ACT copies and this queue's out-DMAs in
                # dependency order: copy g keyed g, DMA j keyed by its last
                # block + 0.5, so every DMA follows the copies it waits on.
                items = []
                for j, (n, q) in enumerate(out_plan):
                    if q == "act":
                        items.append((dma_first_blk[j] + n - 0.5, "dma", j))
                for g in range(NRB2):
                    if cp_eng[g] == 2:
                        items.append((float(g), "cp", g))
                for _, kind, idx in sorted(items):
                    if kind == "dma":
                        issue_out(scalar, nc.scalar, idx)
                    else:
                        act_copy(scalar, idx)

        nc.compile()
    return nc


def _get_program(scheme):
    if scheme not in _PROGRAM_CACHE:
        if scheme == "bf16s":
            _PROGRAM_CACHE[scheme] = _build_bf16s(
                nwarm=int(os.environ.get("BIATT_NWARM", "24"))
            )
        elif scheme == "raw":
            _PROGRAM_CACHE[scheme] = _build_raw()
        elif scheme == "bf16x2":
            _PROGRAM_CACHE[scheme] = _build_bf16x2()
        else:
            _PROGRAM_CACHE[scheme] = _build_f32(
                mybir.dt.float32r if scheme == "f32r" else _F32
            )
    return _PROGRAM_CACHE[scheme]


def _chunk_pieces(mat_t, dtype, npiece):
    """[K=512, len] -> npiece contiguous [128, 4/npiece, len] partition-major
    K-chunk groups."""
    ln = mat_t.shape[1]
    c = np.ascontiguousarray(
        mat_t.reshape(KC, P, ln).transpose(1, 0, 2).astype(dtype)
    )  # [128, 4, len]
    per = KC // npiece
    return [np.ascontiguousarray(c[:, i * per:(i + 1) * per]) for i in range(npiece)]


def _chunk_halves(mat_t, dtype):
    return _chunk_pieces(mat_t, dtype, 2)


def _split_hi_lo(a):
    hi = a.astype(ml_dtypes.bfloat16)
    lo = (a - hi.astype(np.float32)).astype(ml_dtypes.bfloat16)
    return hi, lo


def kernel(**inputs):
    global _LAST_EXEC_NS

    atoms = np.ascontiguousarray(np.asarray(inputs["atoms_vector"], dtype=np.float32))
    amino = np.ascontiguousarray(np.asarray(inputs["amino_vector"], dtype=np.float32))
    Wcc = np.asarray(inputs["Wcc"], dtype=np.float32)
    Wcp = np.asarray(inputs["Wcp"], dtype=np.float32)
    bcc = np.asarray(inputs["bcc"], dtype=np.float32)
    bcp = np.asarray(inputs["bcp"], dtype=np.float32)

    # Fold the four weight blocks (concat([v]*4, 1) @ W == v @ sum-of-blocks).
    wcc_f = Wcc.reshape(4, D, D).sum(axis=0)
    wcp_f = Wcp.reshape(4, D, D).sum(axis=0)

    scheme = os.environ.get("BIATT_MM", "bf16s")
    nc = _get_program(scheme)

    in_maps = []
    if scheme == "bf16s":
        # Stream-split sharding: cores 0-3 compute cf rows (atoms @ wcc_f),
        # cores 4-7 pf rows (amino @ wcp_f); 1536 rows per core.
        w_bf = {
            "cc": wcc_f.astype(ml_dtypes.bfloat16),
            "cp": wcp_f.astype(ml_dtypes.bfloat16),
        }
        for c in range(N_CORES):
            stream = "cc" if c < 4 else "cp"
            base = atoms if c < 4 else amino
            sl = slice((c % 4) * SHARD2, (c % 4 + 1) * SHARD2)
            xt = base[sl].T.astype(ml_dtypes.bfloat16)  # [512, 1536]
            m = {}
            wb = w_bf[stream]
            xb = np.empty((P, 2, 2, XH), dtype=ml_dtypes.bfloat16)
            for k in range(KC):
                chunk = xt[k * P:(k + 1) * P]
                if k == 0:
                    p0a = np.empty((P, D + 2 * P), dtype=ml_dtypes.bfloat16)
                    p0a[:, :D] = wb[:P]
                    p0a[:, D:] = chunk[:, :2 * P]
                    m["p0a"] = p0a
                    m["p0b"] = np.ascontiguousarray(chunk[:, 2 * P:XH])
                elif k == 3:
                    p3a = np.empty((P, D + 3 * P), dtype=ml_dtypes.bfloat16)
                    p3a[:, :D] = wb[k * P:(k + 1) * P]
                    p3a[:, D:] = chunk[:, :3 * P]
                    m["p3a"] = p3a
                    m["p3b"] = np.ascontiguousarray(chunk[:, 3 * P:XH])
                else:
                    pk = np.empty((P, D + XH), dtype=ml_dtypes.bfloat16)
                    pk[:, :D] = wb[k * P:(k + 1) * P]
                    pk[:, D:] = chunk[:, :XH]
                    m[f"p{k}"] = pk
                xb[:, k // 2, k % 2] = chunk[:, XH:]
            m["xb01"] = np.ascontiguousarray(xb[:, 0])
            m["xb23"] = np.ascontiguousarray(xb[:, 1])
            in_maps.append(m)
    elif scheme in ("bf16x2", "raw"):
        # raw: wcch/xh in four per-chunk pieces, the rest in two halves;
        # tile bf16x2: everything in two halves.
        n_first = 2
        wcch, wccl = _split_hi_lo(wcc_f)
        wcph, wcpl = _split_hi_lo(wcp_f)
        w_parts = {}
        for nm, arr, npiece in (("wcch", wcch, n_first), ("wccl", wccl, 2),
                                ("wcph", wcph, 2), ("wcpl", wcpl, 2)):
            for i, p in enumerate(_chunk_pieces(arr, ml_dtypes.bfloat16, npiece)):
                w_parts[f"{nm}{i}"] = p
        for c in range(N_CORES):
            sl = slice(c * SHARD, (c + 1) * SHARD)
            m = dict(w_parts)
            for nm, base in (("x", atoms), ("y", amino)):
                t = base[sl].T  # [512, 768]
                hi, lo = _split_hi_lo(t)
                nh = n_first if nm == "x" else 2
                for i, p in enumerate(_chunk_pieces(hi, ml_dtypes.bfloat16, nh)):
                    m[f"{nm}h{i}"] = p
                for i, p in enumerate(_chunk_pieces(lo, ml_dtypes.bfloat16, 2)):
                    m[f"{nm}l{i}"] = p
            in_maps.append(m)
    else:
        w_parts = {}
        for nm, arr in (("wcc", wcc_f), ("wcp", wcp_f)):
            w_parts[f"{nm}0"], w_parts[f"{nm}1"] = _chunk_halves(arr, np.float32)
        for c in range(N_CORES):
            sl = slice(c * SHARD, (c + 1) * SHARD)
            m = dict(w_parts)
            m["x0"], m["x1"] = _chunk_halves(atoms[sl].T, np.float32)
            m["y0"], m["y1"] = _chunk_halves(amino[sl].T, np.float32)
            in_maps.append(m)

    trace = bool(os.environ.get("BIATT_TRACE"))
    try:
        res = run_bass_kernel_spmd(nc, in_maps, list(range(N_CORES)), trace=trace)
    except Exception:
        # One retry: a transiently wedged NeuronCore surfaces as a runtime
        # error on an otherwise-valid program.
        res = run_bass_kernel_spmd(nc, in_maps, list(range(N_CORES)), trace=trace)
    _LAST_EXEC_NS = res.exec_time_ns

    if scheme == "bf16s":
        def _unpack(c):
            # Device layout [128, 12, 512] (partition-major) -> [1536, 512].
            o = res.results[c]["out"]
            return o.transpose(1, 0, 2).reshape(SHARD2, D).astype(np.float32)

        cf = np.concatenate([_unpack(c) for c in range(4)], axis=0)
        pf = np.concatenate([_unpack(c) for c in range(4, 8)], axis=0)
    else:
        cf = np.concatenate(
            [res.results[c]["cf"].reshape(SHARD, D) for c in range(N_CORES)],
            axis=0,
        )
        pf = np.concatenate(
            [res.results[c]["pf"].reshape(SHARD, D) for c in range(N_CORES)],
            axis=0,
        )
    cf += bcc  # rank-1 epilogue on the gathered output
    pf += bcp
    return cf, pf



# revision 43
# speedup vs baseline: 3.0014x; 1.0081x over previous
"""BiATT kernel for 8 Trainium2 NeuronCores.

The reference module's bilinear-attention branch is dead code: the
"attention" weights are softmax(axis=1) over [N, 1] tensors, which is
exactly 1.0 for every row.  Hence

    cf_final = atoms_vector @ (Wcc[0:D] + Wcc[D:2D] + Wcc[2D:3D] + Wcc[3D:4D]) + bcc
    pf_final = amino_vector @ (Wcp[0:D] + Wcp[D:2D] + Wcp[2D:3D] + Wcp[3D:4D]) + bcp

bit-for-bit up to fp32 rounding, i.e. two [6144,512] @ [512,512] matmuls
with host-folded weights.

Default scheme "bf16s": stream-split sharding (cores 0-3 compute cf rows,
cores 4-7 pf rows, 1536 rows per core) with a SINGLE bf16 matmul term and
bf16 outputs, converted to f32 on the host.  Measured end-to-end error is
~3.5e-3 against the fp32 reference (the harness gate is 2e-2); per-core
traffic is 2.05 MB in + 1.57 MB out and 48 matmuls (805 MFLOP) -- one
third of the PE work and a quarter of the DMA bytes of the bf16x2 path.
See _build_bf16s for the pipeline schedule.  BIATT_MM selects the legacy
schemes (raw = hand-scheduled bf16x2, bf16x2 = Tile bf16x2, f32, f32r);
BIATT_NWARM tunes the warm-up burst.

The bias is added on the host during the gather (a rank-1 epilogue on the
full output).
"""

import os

import ml_dtypes
import numpy as np

import concourse.bacc as bacc
import concourse.bass as bass
import concourse.mybir as mybir
import concourse.tile as tile
from concourse.bass_utils import run_bass_kernel_spmd

N_CORES = 8
D = 512          # feature dim
N_ROWS = 6144    # rows of atoms_vector / amino_vector
SHARD = N_ROWS // N_CORES   # 768 rows per core
P = 128          # SBUF partitions
KC = D // P      # 4 contraction chunks
NRB = SHARD // P  # 6 row blocks per shard

_F32 = mybir.dt.float32
_BF16 = mybir.dt.bfloat16
_PROGRAM_CACHE = {}

_LAST_EXEC_NS = None


def _new_bass():
    return bacc.Bacc(
        "TRN2",
        target_bir_lowering=False,
        debug=False,
        num_devices=N_CORES,
    )


def _build_bf16x2():
    """Split-bf16 path: per stream (cc / cp) the activation comes as hi/lo
    bf16 halves and the folded weight as hi/lo bf16 halves.  Input tensors
    are partition-major K-chunked ([128, nk, len]) so each is one large
    contiguous DMA.  psum[rb] accumulates 12 matmuls: k0..3 of xh@wh,
    xl@wh, xh@wl.

    Perf structure: inputs are two-chunk halves loaded in consumption order
    on the Sync HWDGE ring (output DMAs ride the Activation ring so the two
    dispatch streams never serialize against each other); a burst of
    throwaway matmuls on scratch tiles keeps the PE busy during the DMA
    lead so the HAM clock gate is released (2.4 GHz) when the real matmul
    stream starts."""
    nc = _new_bass()

    # names: {tensor}{piece}; each tensor comes as 2 two-chunk halves.
    d = {}
    layout = {}
    for t, ln, npiece, nk in (
        ("xh", SHARD, 2, 2), ("wcch", D, 2, 2),
        ("xl", SHARD, 2, 2), ("wccl", D, 2, 2),
        ("yh", SHARD, 2, 2), ("wcph", D, 2, 2),
        ("yl", SHARD, 2, 2), ("wcpl", D, 2, 2),
    ):
        layout[t] = (ln, npiece, nk)
        for h in range(npiece):
            d[f"{t}{h}"] = nc.dram_tensor(
                f"{t}{h}", [P, nk, ln], _BF16, kind="ExternalInput"
            ).ap()

    cf = nc.dram_tensor("cf", [NRB, P, D], _F32, kind="ExternalOutput").ap()
    pf = nc.dram_tensor("pf", [NRB, P, D], _F32, kind="ExternalOutput").ap()

    with tile.TileContext(nc) as tc:
        with (
            tc.tile_pool(name="ins", bufs=1) as ins,
            tc.tile_pool(name="warm", bufs=1) as warm,
            tc.tile_pool(name="psum", bufs=7, space=bass.MemorySpace.PSUM) as psum,
            tc.tile_pool(name="wpsum", bufs=1, space=bass.MemorySpace.PSUM) as wpsum,
            tc.tile_pool(name="outs", bufs=8) as outs,
        ):
            # PE warm-up: ~4us of dependency-free matmuls on scratch data,
            # issued while the input DMAs stream in.  Keeps the HAM activity
            # window busy so the real matmuls run at 2.4 GHz from the start.
            wsrc = warm.tile([P, 2 * P], _BF16, tag="wsrc")
            nc.gpsimd.memset(wsrc[:], 0.0)
            wps = wpsum.tile([P, P], _F32, tag="wps")
            for i in range(40):
                nc.tensor.matmul(
                    wps[:], wsrc[:, 0:P], wsrc[:, P:2 * P],
                    start=(i == 0), stop=(i == 39),
                )

            # Load order == consumption order (cf stream first).
            s = {}
            def load(engine, name):
                ln, npiece, nk = layout[name[:-1]]
                t = ins.tile([P, nk, ln], _BF16, tag=name)
                engine.dma_start(t[:], d[name][:])
                s[name] = t

            for name in ("wcch0", "xh0", "wcch1", "xh1",
                         "xl0", "xl1", "wccl0", "wccl1",
                         "wcph0", "yh0", "wcph1", "yh1",
                         "yl0", "yl1", "wcpl0", "wcpl1"):
                load(nc.sync, name)

            def piece(t, k):
                ln, npiece, nk = layout[t]
                return s[f"{t}{k // nk}"][:, k % nk, :]

            for a, w, out_d in (("x", "wcc", cf), ("y", "wcp", pf)):
                for rb in range(NRB):
                    ps = psum.tile([P, D], _F32, tag="ps")
                    idx = 0
                    for ah, wh2 in ((f"{a}h", f"{w}h"), (f"{a}l", f"{w}h"),
                                    (f"{a}h", f"{w}l")):
                        for k in range(KC):
                            nc.tensor.matmul(
                                ps[:],
                                piece(ah, k)[:, rb * P:(rb + 1) * P],
                                piece(wh2, k),
                                start=(idx == 0),
                                stop=(idx == 3 * KC - 1),
                            )
                            idx += 1
                    ot = outs.tile([P, D], _F32, tag="ot")
                    nc.vector.tensor_copy(ot[:], ps[:])
                    nc.scalar.dma_start(out_d[rb], ot[:])

    nc.compile()
    return nc


_IN_ORDER = ("wcch0", "xh0", "wcch1", "xh1", "xl0", "xl1", "wccl0", "wccl1",
             "wcph0", "yh0", "wcph1", "yh1", "yl0", "yl1", "wcpl0", "wcpl1")

# ---------------------------------------------------------------------------
# Single-term bf16 scheme ("bf16s"): one stream per core (cores 0-3 compute
# cf rows, 4-7 pf rows; 1536 rows each), single bf16 matmul term (measured
# end-to-end error ~3.4e-3 vs the 2e-2 gate), bf16 outputs converted to f32
# on the host.  Per-core traffic: 2.05 MB in + 1.57 MB out vs the bf16x2
# path's 8.4 MB; PE work 48 matmuls (805 MFLOP) vs 144.
# ---------------------------------------------------------------------------

SHARD2 = N_ROWS // 4      # 1536 rows per core (4 cores per stream)
NRB2 = SHARD2 // P        # 12 row blocks
XH = SHARD2 // 2          # 768 columns per xT half (row-blocks 0-5 / 6-11)


# Output DMA plan: (blocks, queue) per DMA.  Early big pieces go on the
# Activation queue; the late pieces spread across queues so their
# HWDGE/SWDGE dispatch chains overlap.  "pool" DMAs use the SWDGE path,
# which does not occupy the shared HWDGE generator at all.
OUT_PLAN = ((3, "act"), (3, "act"), (2, "act"), (2, "act"), (1, "sync"),
            (1, "sync"))
OUT_SPLIT = tuple(n for n, _ in OUT_PLAN)


def _build_bf16s(nwarm=30, out_plan=OUT_PLAN):
    """Hand-scheduled raw pipeline, one [1536,512]@[512,512] bf16 matmul.

    DMA dispatch is the scarce resource (each HWDGE dispatch occupies the
    shared generator ~630ns, and the first DMA's SEQ+HWDGE+DGE lead is
    ~1.9us), so inputs are 7 DMAs: p0a/p0b split the k0 chunk (weight
    [128,512] packed with activation row-blocks 0-2 / 3-5) so the first
    matmul gates on 224KB; p1..p3 pack weight chunk k with activation
    chunk k of row-blocks 0-5; xb01/xb23 carry the activation chunks of
    row-blocks 6-11 two-at-a-time.  Outputs: row-blocks 0-10 are
    f32->bf16 copied (alternating DVE/GPSIMD, whose 658/806ns per block
    matches the 426ns stop stagger) into 4 private SBUF slots and stored
    by 4 Activation-queue DMAs (3/3/3/2 blocks, partition-major
    [128,12,512] DRAM, host re-transposes); block 11 is DMA'd from its
    PSUM bank as f32 on the idle sync queue, skipping the copy.

    The warm-up matmuls run on uninitialized SBUF from t~0 (their PSUM
    bank is reset by a later start=True) purely to hold the PE p-state at
    full clock before the real stream starts.

    Groups 0-5 (banks 0-5): k0,k1 k-outer, then per-rb (k2,k3) so stops
    stagger 426ns; groups 6-11 (banks 6,7,0-3) mirror, bank 0-3 reuse
    gated on the first-half copies."""
    from contextlib import ExitStack

    nc = _new_bass()

    d = {}
    d["p0a"] = nc.dram_tensor("p0a", [P, D + 2 * P], _BF16, kind="ExternalInput").ap()
    d["p0b"] = nc.dram_tensor("p0b", [P, 4 * P], _BF16, kind="ExternalInput").ap()
    for k in range(1, KC - 1):
        d[f"p{k}"] = nc.dram_tensor(
            f"p{k}", [P, D + XH], _BF16, kind="ExternalInput").ap()
    d["p3a"] = nc.dram_tensor("p3a", [P, D + P], _BF16, kind="ExternalInput").ap()
    d["p3b"] = nc.dram_tensor("p3b", [P, 5 * P], _BF16, kind="ExternalInput").ap()
    d["xb01"] = nc.dram_tensor("xb01", [P, 2, XH], _BF16, kind="ExternalInput").ap()
    d["xb23"] = nc.dram_tensor("xb23", [P, 2, XH], _BF16, kind="ExternalInput").ap()
    out_d = nc.dram_tensor("out", [P, NRB2, D], _BF16, kind="ExternalOutput").ap()

    out_split = tuple(n for n, _ in out_plan)
    DMA_ORDER = ("p0a", "p0b", "p1", "p2", "p3a", "p3b", "xb01", "xb23")
    SHAPES = {"p0a": [P, D + 2 * P], "p0b": [P, 4 * P],
              "p1": [P, D + XH], "p2": [P, D + XH],
              "p3a": [P, D + P], "p3b": [P, 5 * P],
              "xb01": [P, 2, XH], "xb23": [P, 2, XH]}
    NJ = len(out_split)

    with ExitStack() as ctx:
        sb = {
            name: ctx.enter_context(
                nc.sbuf_tensor(f"sb_{name}", SHAPES[name], _BF16))
            for name in DMA_ORDER
        }
        # One private slot per output DMA -- no ping-pong waits anywhere.
        max_n = max(out_split)
        outsb = ctx.enter_context(
            nc.sbuf_tensor("outsb", [P, NJ, max_n * D], _BF16))
        warm = ctx.enter_context(nc.sbuf_tensor("warmsb", [P, 2 * P], _BF16))
        ps = [
            ctx.enter_context(nc.psum_tensor(f"psum{i}", [P, D], _F32))
            for i in range(8)
        ]
        s_mm = ctx.enter_context(nc.semaphore("s_mm"))
        s_cpv = ctx.enter_context(nc.semaphore("s_cpv"))  # DVE copies
        s_cpg = ctx.enter_context(nc.semaphore("s_cpg"))  # GPSIMD copies
        s_cpa = ctx.enter_context(nc.semaphore("s_cpa"))  # ACT copies
        s_od = ctx.enter_context(nc.semaphore("s_od"))    # out-DMA completions
        gates = {
            name: ctx.enter_context(nc.semaphore(f"s_{name}"))
            for name in DMA_ORDER
        }

        blk_dma = []
        for j, n in enumerate(out_split):
            for o in range(n):
                blk_dma.append((j, o))
        dma_first_blk = [sum(out_split[:j]) for j in range(NJ)]
        # 0 = DVE, 1 = GPSIMD, 2 = ACT (unused by default).
        cp_eng = [g % 2 for g in range(NRB2)]
        cp_sem_of = {0: s_cpv, 1: s_cpg, 2: s_cpa}

        def cp_counts(last_blk):
            return tuple(
                sum(1 for g in range(NRB2) if cp_eng[g] == e and g <= last_blk)
                for e in range(3)
            )

        def wchunk(k):
            if k == 0:
                return sb["p0a"][:, 0:D]
            if k == 3:
                return sb["p3a"][:, 0:D]
            return sb[f"p{k}"][:, 0:D]

        def xblk(half, k, rb):
            if half == "a":
                if k == 0:
                    t, r = ("p0a", rb) if rb < 2 else ("p0b", rb - 2)
                    off = D if t == "p0a" else 0
                    return sb[t][:, off + r * P:off + (r + 1) * P]
                if k == 3:
                    t, r = ("p3a", rb) if rb < 1 else ("p3b", rb - 1)
                    off = D if t == "p3a" else 0
                    return sb[t][:, off + r * P:off + (r + 1) * P]
                return sb[f"p{k}"][:, D + rb * P:D + (rb + 1) * P]
            return sb["xb01" if k < 2 else "xb23"][:, k % 2, rb * P:(rb + 1) * P]

        def copy_loop(eng_idx, engine, eng_ns, sem):
            for g in range(NRB2):
                if cp_eng[g] != eng_idx:
                    continue
                j, o = blk_dma[g]
                engine.wait_ge(s_mm, g + 1)
                eng_ns.tensor_copy(
                    outsb[:, j, o * D:(o + 1) * D], ps[g % 8][:]
                ).then_inc(sem, 1)

        with nc.Block() as block:

            def issue_out(handle, eng_ns, j):
                n = out_split[j]
                b0 = dma_first_blk[j]
                counts = cp_counts(b0 + n - 1)
                for e, cnt in enumerate(counts):
                    if cnt:
                        handle.wait_ge(cp_sem_of[e], cnt)
                eng_ns.dma_start(
                    out_d[:, b0:b0 + n, :], outsb[:, j, 0:n * D]
                ).then_inc(s_od, 16)

            def act_copy(scalar, g):
                j, o = blk_dma[g]
                scalar.wait_ge(s_mm, g + 1)
                nc.scalar.activation(
                    outsb[:, j, o * D:(o + 1) * D], ps[g % 8][:],
                    mybir.ActivationFunctionType.Copy,
                ).then_inc(s_cpa, 1)

            @block.sync
            def _(sync):
                for name in DMA_ORDER:
                    if name in ("p0b", "xb23"):
                        continue  # issued on the Pool/SWDGE queue
                    sync.dma_start(sb[name][:], d[name][:]).then_inc(
                        gates[name], 16
                    )
                for j, (n, q) in enumerate(out_plan):
                    if q == "sync":
                        issue_out(sync, nc.sync, j)

            @block.vector
            def _(vector):
                copy_loop(0, vector, nc.vector, s_cpv)
                for j, (n, q) in enumerate(out_plan):
                    if q == "vec":
                        issue_out(vector, nc.vector, j)

            @block.gpsimd
            def _(gpsimd):
                # p0b rides SWDGE (no shared-HWDGE occupancy) so the sync
                # chain's HWDGE slots all go to the k1..k3 gate pieces;
                # xb23 is consumed last and dispatches only after p1 lands
                # so its transfer does not displace the earlier gates.
                nc.gpsimd.dma_start(sb["p0b"][:], d["p0b"][:]).then_inc(
                    gates["p0b"], 16
                )
                gpsimd.wait_ge(gates["p1"], 16)
                nc.gpsimd.dma_start(sb["xb23"][:], d["xb23"][:]).then_inc(
                    gates["xb23"], 16
                )
                for j, (n, q) in enumerate(out_plan):
                    if q == "pool":
                        issue_out(gpsimd, nc.gpsimd, j)
                for j, (n, q) in enumerate(out_plan):
                    if q == "pool":
                        issue_out(gpsimd, nc.gpsimd, j)

            @block.tensor
            def _(tensor):
                for i in range(nwarm):
                    nc.tensor.matmul(
                        ps[7][:, 0:P], warm[:, 0:P], warm[:, P:2 * P],
                        start=(i == 0), stop=(i == nwarm - 1),
                    )
                waited = set()

                def gate(name):
                    if name not in waited:
                        waited.add(name)
                        tensor.wait_ge(gates[name], 16)

                def bank_wait(g):
                    if g >= 8:
                        e = cp_eng[g - 8]
                        tensor.wait_ge(cp_sem_of[e], cp_counts(g - 8)[e])

                def mm(half, k, rb, start, stop):
                    g = (0 if half == "a" else 6) + rb
                    if start:
                        bank_wait(g)
                    m = nc.tensor.matmul(
                        ps[g % 8][:], xblk(half, k, rb), wchunk(k),
                        start=start, stop=stop,
                    )
                    if stop:
                        m.then_inc(s_mm, 1)

                # Phase a: the first gate blocks the PE SEQ, and the ~5
                # instructions issued after any blocking wait run at the
                # mid p-state -- burn them on short throwaway matmuls so
                # every real matmul runs at full clock.
                gate("p0a")
                for i in range(5):
                    nc.tensor.matmul(
                        ps[7][:, 0:32], warm[:, 0:P], warm[:, P:P + 32],
                        start=(i == 0), stop=(i == 4),
                    )
                for rb in range(6):
                    if rb == 2:
                        gate("p0b")
                    mm("a", 0, rb, True, False)
                gate("p1")
                for rb in range(6):
                    mm("a", 1, rb, False, False)
                gate("p2")
                gate("p3a")
                for rb in range(6):
                    if rb == 1:
                        gate("p3b")
                    mm("a", 2, rb, False, False)
                    mm("a", 3, rb, False, True)
                # Phase b: k0 k-outer; per-rb (k1,k2,k3) so stops stagger
                # 639ns apart, matching the DVE copy throughput.
                gate("xb01")
                for rb in range(6):
                    mm("b", 0, rb, True, False)
                for rb in range(6):
                    mm("b", 1, rb, False, False)
                    if rb == 0:
                        gate("xb23")
                    mm("b", 2, rb, False, False)
                    mm("b", 3, rb, False, True)

            @block.scalar
            def _(scalar):
                # Dummy activation so the Copy act-table is loaded long
                # before the first real copy.
                nc.scalar.activation(
                    outsb[:, 0, 0:32], warm[:, 0:32],
                    mybir.ActivationFunctionType.Copy,
                )
                # Interleave ACT copies and this queue's out-DMAs in
                # dependency order: copy g keyed g, DMA j keyed by its last
                # block + 0.5, so every DMA follows the copies it waits on.
                items = []
                for j, (n, q) in enumerate(out_plan):
                    if q == "act":
                        items.append((dma_first_blk[j] + n - 0.5, "dma", j))
                for g in range(NRB2):
                    if cp_eng[g] == 2:
                        items.append((float(g), "cp", g))
                for _, kind, idx in sorted(items):
                    if kind == "dma":
                        issue_out(scalar, nc.scalar, idx)
                    else:
                        act_copy(scalar, idx)

        nc.compile()
    return nc


def _get_program(scheme):
    if scheme not in _PROGRAM_CACHE:
        if scheme == "bf16s":
            _PROGRAM_CACHE[scheme] = _build_bf16s(
                nwarm=int(os.environ.get("BIATT_NWARM", "24"))
            )
        elif scheme == "raw":
            _PROGRAM_CACHE[scheme] = _build_raw()
        elif scheme == "bf16x2":
            _PROGRAM_CACHE[scheme] = _build_bf16x2()
        else:
            _PROGRAM_CACHE[scheme] = _build_f32(
                mybir.dt.float32r if scheme == "f32r" else _F32
            )
    return _PROGRAM_CACHE[scheme]


def _chunk_pieces(mat_t, dtype, npiece):
    """[K=512, len] -> npiece contiguous [128, 4/npiece, len] partition-major
    K-chunk groups."""
    ln = mat_t.shape[1]
    c = np.ascontiguousarray(
        mat_t.reshape(KC, P, ln).transpose(1, 0, 2).astype(dtype)
    )  # [128, 4, len]
    per = KC // npiece
    return [np.ascontiguousarray(c[:, i * per:(i + 1) * per]) for i in range(npiece)]


def _chunk_halves(mat_t, dtype):
    return _chunk_pieces(mat_t, dtype, 2)


def _split_hi_lo(a):
    hi = a.astype(ml_dtypes.bfloat16)
    lo = (a - hi.astype(np.float32)).astype(ml_dtypes.bfloat16)
    return hi, lo


def kernel(**inputs):
    global _LAST_EXEC_NS

    atoms = np.ascontiguousarray(np.asarray(inputs["atoms_vector"], dtype=np.float32))
    amino = np.ascontiguousarray(np.asarray(inputs["amino_vector"], dtype=np.float32))
    Wcc = np.asarray(inputs["Wcc"], dtype=np.float32)
    Wcp = np.asarray(inputs["Wcp"], dtype=np.float32)
    bcc = np.asarray(inputs["bcc"], dtype=np.float32)
    bcp = np.asarray(inputs["bcp"], dtype=np.float32)

    # Fold the four weight blocks (concat([v]*4, 1) @ W == v @ sum-of-blocks).
    wcc_f = Wcc.reshape(4, D, D).sum(axis=0)
    wcp_f = Wcp.reshape(4, D, D).sum(axis=0)

    scheme = os.environ.get("BIATT_MM", "bf16s")
    nc = _get_program(scheme)

    in_maps = []
    if scheme == "bf16s":
        # Stream-split sharding: cores 0-3 compute cf rows (atoms @ wcc_f),
        # cores 4-7 pf rows (amino @ wcp_f); 1536 rows per core.
        w_bf = {
            "cc": wcc_f.astype(ml_dtypes.bfloat16),
            "cp": wcp_f.astype(ml_dtypes.bfloat16),
        }
        for c in range(N_CORES):
            stream = "cc" if c < 4 else "cp"
            base = atoms if c < 4 else amino
            sl = slice((c % 4) * SHARD2, (c % 4 + 1) * SHARD2)
            xt = base[sl].T.astype(ml_dtypes.bfloat16)  # [512, 1536]
            m = {}
            wb = w_bf[stream]
            xb = np.empty((P, 2, 2, XH), dtype=ml_dtypes.bfloat16)
            for k in range(KC):
                chunk = xt[k * P:(k + 1) * P]
                if k == 0:
                    p0a = np.empty((P, D + 2 * P), dtype=ml_dtypes.bfloat16)
                    p0a[:, :D] = wb[:P]
                    p0a[:, D:] = chunk[:, :2 * P]
                    m["p0a"] = p0a
                    m["p0b"] = np.ascontiguousarray(chunk[:, 2 * P:XH])
                elif k == 3:
                    p3a = np.empty((P, D + P), dtype=ml_dtypes.bfloat16)
                    p3a[:, :D] = wb[k * P:(k + 1) * P]
                    p3a[:, D:] = chunk[:, :P]
                    m["p3a"] = p3a
                    m["p3b"] = np.ascontiguousarray(chunk[:, P:XH])
                else:
                    pk = np.empty((P, D + XH), dtype=ml_dtypes.bfloat16)
                    pk[:, :D] = wb[k * P:(k + 1) * P]
                    pk[:, D:] = chunk[:, :XH]
                    m[f"p{k}"] = pk
                xb[:, k // 2, k % 2] = chunk[:, XH:]
            m["xb01"] = np.ascontiguousarray(xb[:, 0])
            m["xb23"] = np.ascontiguousarray(xb[:, 1])
            in_maps.append(m)
    elif scheme in ("bf16x2", "raw"):
        # raw: wcch/xh in four per-chunk pieces, the rest in two halves;
        # tile bf16x2: everything in two halves.
        n_first = 2
        wcch, wccl = _split_hi_lo(wcc_f)
        wcph, wcpl = _split_hi_lo(wcp_f)
        w_parts = {}
        for nm, arr, npiece in (("wcch", wcch, n_first), ("wccl", wccl, 2),
                                ("wcph", wcph, 2), ("wcpl", wcpl, 2)):
            for i, p in enumerate(_chunk_pieces(arr, ml_dtypes.bfloat16, npiece)):
                w_parts[f"{nm}{i}"] = p
        for c in range(N_CORES):
            sl = slice(c * SHARD, (c + 1) * SHARD)
            m = dict(w_parts)
            for nm, base in (("x", atoms), ("y", amino)):
                t = base[sl].T  # [512, 768]
                hi, lo = _split_hi_lo(t)
                nh = n_first if nm == "x" else 2
                for i, p in enumerate(_chunk_pieces(hi, ml_dtypes.bfloat16, nh)):
                    m[f"{nm}h{i}"] = p
                for i, p in enumerate(_chunk_pieces(lo, ml_dtypes.bfloat16, 2)):
                    m[f"{nm}l{i}"] = p
            in_maps.append(m)
    else:
        w_parts = {}
        for nm, arr in (("wcc", wcc_f), ("wcp", wcp_f)):
            w_parts[f"{nm}0"], w_parts[f"{nm}1"] = _chunk_halves(arr, np.float32)
        for c in range(N_CORES):
            sl = slice(c * SHARD, (c + 1) * SHARD)
            m = dict(w_parts)
            m["x0"], m["x1"] = _chunk_halves(atoms[sl].T, np.float32)
            m["y0"], m["y1"] = _chunk_halves(amino[sl].T, np.float32)
            in_maps.append(m)

    trace = bool(os.environ.get("BIATT_TRACE"))
    try:
        res = run_bass_kernel_spmd(nc, in_maps, list(range(N_CORES)), trace=trace)
    except Exception:
        # One retry: a transiently wedged NeuronCore surfaces as a runtime
        # error on an otherwise-valid program.
        res = run_bass_kernel_spmd(nc, in_maps, list(range(N_CORES)), trace=trace)
    _LAST_EXEC_NS = res.exec_time_ns

    if scheme == "bf16s":
        def _unpack(c):
            # Device layout [128, 12, 512] (partition-major) -> [1536, 512].
            o = res.results[c]["out"]
            return o.transpose(1, 0, 2).reshape(SHARD2, D).astype(np.float32)

        cf = np.concatenate([_unpack(c) for c in range(4)], axis=0)
        pf = np.concatenate([_unpack(c) for c in range(4, 8)], axis=0)
    else:
        cf = np.concatenate(
            [res.results[c]["cf"].reshape(SHARD, D) for c in range(N_CORES)],
            axis=0,
        )
        pf = np.concatenate(
            [res.results[c]["pf"].reshape(SHARD, D) for c in range(N_CORES)],
            axis=0,
        )
    cf += bcc  # rank-1 epilogue on the gathered output
    pf += bcp
    return cf, pf

